# revision 89
# baseline (speedup 1.0000x reference)
"""Trainium2 Bass kernel for a full causal MHA layer (B=2, T=2048, C=2048, H=16,
partial RoPE on first 64 dims of each 128-dim head).

Sharding over 8 cores: core c handles batch b=c//4 and heads [4g, 4g+4), g=c%4
(tensor-parallel over heads x data-parallel over batch).

Fully fused single pass per core, fp16 data plane (fp32 PSUM accumulation):
  for each 512-token chunk ic:
    proj q/k (fp16 weights stationary, fp16 x moving), bias+partial-RoPE,
      q and k stay resident in SBUF (no DRAM spills)
    proj v -> v_res [key, jt, m] fp16 resident
    attention for chunk ic over heads h: per key-tile jt
      scoresT[k,q] (k_res stationary fp16, q moving fp16)
      -> exp(scale*s - 10*ln2) -> ex fp16 (Act), triangle mask on diagonal
         tiles only (DVE mult by a const [128,128] triu mask), exact causal
         col-trimming
      -> av accumulation outT[d,q] via PE; softmax denominator via DVE
         exsum adds + gpsimd partition_all_reduce (no PE ones-matmuls)
      output-projection matmuls of chunk ic-1 are woven between attention
      matmuls to keep PE busy during Act-latency windows
    phase3(ic): out partial outT[c,q] = sum_mt woT attn, DVE evict fp16, DMA
Host: slices inputs per core (fp16), sums the 4 TP partials per batch + bo.
"""

import math

import ml_dtypes
import numpy as np

NPF8 = ml_dtypes.float8_e4m3

import concourse.bass_isa as bass_isa
import concourse.mybir as mybir
import concourse.tile as tile
from concourse import bacc
from concourse.bass_utils import run_bass_kernel_spmd

F32 = mybir.dt.float32
F16 = mybir.dt.float16
F8 = mybir.dt.float8e4

B, T, C = 2, 2048, 2048
H = 16
HS = 128
ROT = 64
HALF = 32
BASE = 10000.0

N_CORES = 8
TPG = 4                # TP group size (heads split)
H_LOC = H // TPG       # 4 heads per core
M = H_LOC * HS         # 512 local head-dim columns
SCALE = 1.0 / math.sqrt(HS)
EXP_BIAS = -10.0 * math.log(2.0)   # exp(s*SCALE - 10ln2): keeps fp16 in range
SX = 16.0                 # fp8 quantization scale for x
SW = 1024.0               # fp8 quantization scale for Wq/Wk/Wv
INV_S = 1.0 / (SX * SW)   # folded into the projection evictions
SWO = 1024.0              # fp8 quantization scale for Wo
SA = 64.0                 # fp8 quantization scale for attn outputs
INV_SO = 1.0 / (SWO * SA)  # folded into the output-proj evictions

P = 128
NT = T // 512          # 4 t-chunks of 512
CT = C // P            # 16 contraction tiles
JT = T // P            # 16 key tiles per head

_NC_CACHE = {}
PHASE_MARKS = []  # (label, last-emitted instruction name); debug aid only


def _build(phases=(1, 2, 3)):
    nc = bacc.Bacc(None, target_bir_lowering=False)
    PHASE_MARKS.clear()

    def _mark(label):
        # consumes one instruction name as a monotonic position marker
        PHASE_MARKS.append((label, nc.get_next_instruction_name()))

    xh8 = nc.declare_dram_parameter("xh8", [C, T], F8, isOutput=False)
    xh8r = nc.declare_dram_parameter("xh8r", [C, T], F8, isOutput=False)
    wq8 = nc.declare_dram_parameter("wq8", [C, M], F8, isOutput=False)
    wq8r = nc.declare_dram_parameter("wq8r", [C, M], F8, isOutput=False)
    wk8 = nc.declare_dram_parameter("wk8", [C, M], F8, isOutput=False)
    wk8r = nc.declare_dram_parameter("wk8r", [C, M], F8, isOutput=False)
    wv8 = nc.declare_dram_parameter("wv8", [C, M], F8, isOutput=False)
    wv8r = nc.declare_dram_parameter("wv8r", [C, M], F8, isOutput=False)
    wo8T = nc.declare_dram_parameter("wo8T", [M, C], F8, isOutput=False)
    wo8rT = nc.declare_dram_parameter("wo8rT", [M, C], F8, isOutput=False)
    bqc = nc.declare_dram_parameter("bqc", [P, H_LOC], F32, isOutput=False)
    bkc = nc.declare_dram_parameter("bkc", [P, H_LOC], F32, isOutput=False)
    bvr = nc.declare_dram_parameter("bvr", [1, M], F16, isOutput=False)
    ebias = nc.declare_dram_parameter("ebias", [P, 1], F32, isOutput=False)
    trimask = nc.declare_dram_parameter("trimask", [P, P], F16, isOutput=False)
    cosT = nc.declare_dram_parameter("cosT", [ROT, T], F16, isOutput=False)
    nsT = nc.declare_dram_parameter("nsT", [ROT, T], F16, isOutput=False)
    outT = nc.declare_dram_parameter("outT", [C, T], F16, isOutput=True)

    with tile.TileContext(nc) as tc, \
         tc.tile_pool(name="const", bufs=1) as const, \
         tc.tile_pool(name="xp", bufs=CT) as xpool, \
         tc.tile_pool(name="qc", bufs=2) as qpool, \
         tc.tile_pool(name="at", bufs=2) as atpool, \
         tc.tile_pool(name="rp", bufs=6) as rpool, \
         tc.tile_pool(name="exp", bufs=5) as expool, \
         tc.tile_pool(name="exs", bufs=3) as espool, \
         tc.tile_pool(name="rd", bufs=5) as rdpool, \
         tc.tile_pool(name="a16", bufs=2) as a16pool, \
         tc.tile_pool(name="oe", bufs=3) as oepool, \
         tc.tile_pool(name="psA", bufs=2, space="PSUM") as psA, \
         tc.tile_pool(name="psS", bufs=2, space="PSUM") as psS, \
         tc.tile_pool(name="psV", bufs=2, space="PSUM") as psV:

        cos_sb = const.tile([ROT, T], F16, tag="cos")
        ns_sb = const.tile([ROT, T], F16, tag="ns")
        bq_sb = const.tile([P, H_LOC], F32, tag="bq")
        bk_sb = const.tile([P, H_LOC], F32, tag="bk")
        bvb_sb = const.tile([P, M], F16, tag="bvb")
        eb_sb = const.tile([P, 1], F32, tag="ebias")
        tri_sb = const.tile([P, P], F16, tag="trimask")
        k_res = const.tile([P, H_LOC, T], F16, tag="kres")
        v_res = const.tile([P, JT, M], F16, tag="vres")
        wq_t = [const.tile([P, CT, M], F8, tag=f"wq{i}", name=f"wq{i}")
                for i in range(2)]
        wk_t = [const.tile([P, CT, M], F8, tag=f"wk{i}", name=f"wk{i}")
                for i in range(2)]
        wv_t = [const.tile([P, CT, M], F8, tag=f"wv{i}", name=f"wv{i}")
                for i in range(2)]
        wo_b = const.tile([P, H_LOC, C], F8, tag="wob")
        wor_b = const.tile([P, H_LOC, C], F8, tag="worb")
        wre = [d[:].rearrange("(ct p) m -> p ct m", p=P)
               for d in (wq8, wq8r, wk8, wk8r, wv8, wv8r)]
        wqre, wqrre, wkre, wkrre, wvre, wvrre = wre
        wor = wo8T[:].rearrange("(mt p) c -> p mt c", p=P)
        worr = wo8rT[:].rearrange("(mt p) c -> p mt c", p=P)

        xr = xh8[:].rearrange("(ct p) t -> p ct t", p=P)
        xrr = xh8r[:].rearrange("(ct p) t -> p ct t", p=P)
        otr = outT[:].rearrange("(co p) t -> p co t", p=P)

        def load_x(ic):
            ts0 = ic * 512
            xb = xpool.tile([P, CT, 512], F8, tag="xb8", name=f"xb{ic}", bufs=2)
            xbr = xpool.tile([P, CT, 512], F8, tag="xb8r", name=f"xbr{ic}",
                             bufs=2)
            for j in range(2):
                nc.sync.dma_start(out=xb[:, 8 * j:8 * j + 8, :],
                                  in_=xr[:, 8 * j:8 * j + 8, ts0:ts0 + 512])
                nc.sync.dma_start(out=xbr[:, 8 * j:8 * j + 8, :],
                                  in_=xrr[:, 8 * j:8 * j + 8, ts0:ts0 + 512])
            return xb, xbr

        # startup, ordered by first use: main-term operands (x8 + wq8) first,
        # then the residual streams, then wv (main, res), wk (main, res).
        # x on the SP queue, wq on the Act queue so the two interleave on the
        # DMA engines; chunked so HWDGE overheads don't pace it.
        xb0 = xpool.tile([P, CT, 512], F8, tag="xb8", name="xb0", bufs=2)
        xb0r = xpool.tile([P, CT, 512], F8, tag="xb8r", name="xb0r", bufs=2)
        # first ct-pair as its own small DMA so the very first matmul can
        # start as early as possible
        nc.sync.dma_start(out=xb0[:, 0:2, :], in_=xr[:, 0:2, 0:512])
        nc.scalar.dma_start(out=wq_t[0][:, 0:2, :], in_=wqre[:, 0:2, :])
        nc.sync.dma_start(out=xb0[:, 2:4, :], in_=xr[:, 2:4, 0:512])
        nc.scalar.dma_start(out=wq_t[0][:, 2:4, :], in_=wqre[:, 2:4, :])
        for j in range(1, 4):
            a, b = 4 * j, 4 * j + 4
            nc.sync.dma_start(out=xb0[:, a:b, :], in_=xr[:, a:b, 0:512])
            nc.scalar.dma_start(out=wq_t[0][:, a:b, :], in_=wqre[:, a:b, :])
        for j in range(4):
            a, b = 4 * j, 4 * j + 4
            nc.sync.dma_start(out=xb0r[:, a:b, :], in_=xrr[:, a:b, 0:512])
            nc.scalar.dma_start(out=wq_t[1][:, a:b, :], in_=wqrre[:, a:b, :])
        for j in range(4):
            a, b = 4 * j, 4 * j + 4
            nc.sync.dma_start(out=wv_t[0][:, a:b, :], in_=wvre[:, a:b, :])
        for j in range(4):
            a, b = 4 * j, 4 * j + 4
            nc.sync.dma_start(out=wv_t[1][:, a:b, :], in_=wvrre[:, a:b, :])
        for j in range(4):
            a, b = 4 * j, 4 * j + 4
            nc.sync.dma_start(out=wk_t[0][:, a:b, :], in_=wkre[:, a:b, :])
        for j in range(4):
            a, b = 4 * j, 4 * j + 4
            nc.sync.dma_start(out=wk_t[1][:, a:b, :], in_=wkrre[:, a:b, :])
        ones_sb = const.tile([P, P], F16, tag="ones")
        nc.gpsimd.memset(ones_sb[:], 1.0)
        nc.gpsimd.dma_start(out=bq_sb[:], in_=bqc[:])
        nc.gpsimd.dma_start(out=bk_sb[:], in_=bkc[:])
        nc.gpsimd.dma_start(out=cos_sb[:], in_=cosT[:])
        nc.gpsimd.dma_start(out=ns_sb[:], in_=nsT[:])
        nc.gpsimd.dma_start(out=eb_sb[:], in_=ebias[:])
        nc.gpsimd.dma_start(out=tri_sb[:], in_=trimask[:])
        nc.gpsimd.dma_start(out=bvb_sb[:], in_=bvr[0:1, :].to_broadcast([P, M]))

        NP = CT // 2   # 8 ct-pairs per contraction

        def fb_terms(w_t, xb, xbr):
            """(lhsT_tile, rhs_tile) per error-feedback term: main, w-res,
            x-res. All DoubleRow fp8 over ct-pairs."""
            return ((w_t[0], xb), (w_t[1], xb), (w_t[0], xbr))

        def rope_inplace(dst, tmp_src, ts0, dq=None):
            """dst[0:ROT, 512] fp16 <- rope(tmp_src rows 0:ROT) in place.
            tmp_src rows are pre-rope biased values; dst may alias tmp_src.
            dq picks the DMA queue for the partition-swap (the SP queue is
            backed up with weight loads during chunk 0)."""
            dq = dq or nc.sync
            sh = rpool.tile([ROT, 512], F16, tag="sh")
            dq.dma_start(out=sh[0:HALF], in_=tmp_src[HALF:ROT])
            dq.dma_start(out=sh[HALF:ROT], in_=tmp_src[0:HALF])
            rot = rpool.tile([ROT, 512], F16, tag="rot")
            nc.vector.tensor_tensor(rot[:], sh[:], ns_sb[:, ts0:ts0 + 512],
                                    mybir.AluOpType.mult)
            tcos = rpool.tile([ROT, 512], F16, tag="tcos")
            nc.vector.tensor_tensor(tcos[:], tmp_src[:ROT], cos_sb[:, ts0:ts0 + 512],
                                    mybir.AluOpType.mult)
            nc.vector.tensor_tensor(dst[0:ROT], tcos[:], rot[:],
                                    mybir.AluOpType.add)

        class Ph3:
            """Output projection for chunk ic (fp8 DoubleRow, 3-term error
            feedback); matmuls are dispensed one at a time (step) so they
            weave between attention matmuls. 6 steps per co:
            (term, g) with term in {wo8*a8, wo8r*a8, wo8*a8r}, g the DR pair
            of mt planes."""

            def __init__(self, ic, a8, a8r, pools=None):
                self.ic = ic
                self.terms = ((wo_b, a8), (wor_b, a8), (wo_b, a8r))
                self.items = [(co, s) for co in range(CT) for s in range(6)]
                if ic == NT - 1:
                    # stagger co0/co1: their g0 steps (heads 0-1 only) first,
                    # g1 steps after — bridges the last head's normalize
                    # chain at the attention/finish boundary
                    self.items = (
                        [(0, 0), (0, 1), (0, 2), (1, 0), (1, 1), (1, 2),
                         (0, 3), (0, 4), (0, 5), (1, 3), (1, 4), (1, 5)]
                        + [(co, s) for co in range(2, CT) for s in range(6)])
                self.pos = 0
                self.ps = None
                self.pools = pools or [psA]
                self.finishing = False

            def step(self, n=1):
                for _ in range(n):
                    if self.pos >= len(self.items):
                        return
                    co, s = self.items[self.pos]
                    self.pos += 1
                    # g-major: the three g=0 steps only read heads 0-1,
                    # which are normalized well before heads 2-3 land
                    g, t = divmod(s, 3)
                    if s == 0:
                        pool = self.pools[co % len(self.pools)]
                        ps = pool.tile([P, 512], F32,
                                       tag="psA" if pool is psA else "psV",
                                       name=f"ps3_{self.ic}_{co}")
                        self.ps_by_co = getattr(self, "ps_by_co", {})
                        self.ps_by_co[co] = ps
                    self.ps = self.ps_by_co[co]
                    wt, at = self.terms[t]
                    nc.tensor.matmul(
                        self.ps[:],
                        lhsT=wt[:, 2 * g:2 * g + 2, co * P:(co + 1) * P],
                        rhs=at[:, 2 * g:2 * g + 2, :],
                        start=(s == 0), stop=(s == 5),
                        perf_mode=mybir.MatmulPerfMode.DoubleRow)
                    if s == 5:
                        if co % 4 == 0:
                            self.ot = oepool.tile([P, 4, 512], F16, tag="ot")
                        last = self.ic == NT - 1
                        if self.finishing and last and co % 2 == 0:
                            # end of kernel: DVE is idle, alternate with Act
                            # so the final evict drain is 2-wide
                            with nc.allow_low_precision(reason="fp16 out"):
                                nc.vector.tensor_scalar(
                                    out=self.ot[:, co % 4, :], in0=self.ps[:],
                                    scalar1=INV_SO, scalar2=None,
                                    op0=mybir.AluOpType.mult)
                        elif self.finishing:
                            # post-attention block: DVE is draining attention
                            # tail work, evict on Act
                            nc.scalar.activation(
                                self.ot[:, co % 4, :], self.ps[:],
                                mybir.ActivationFunctionType.Identity,
                                scale=INV_SO)
                        elif co % 3 == 2:
                            nc.scalar.activation(
                                self.ot[:, co % 4, :], self.ps[:],
                                mybir.ActivationFunctionType.Identity,
                                scale=INV_SO)
                        else:
                            # woven between attention matmuls: mostly DVE
                            # with Act picking up every third (gpsimd can't
                            # read PSUM)
                            with nc.allow_low_precision(reason="fp16 out"):
                                nc.vector.tensor_scalar(
                                    out=self.ot[:, co % 4, :], in0=self.ps[:],
                                    scalar1=INV_SO, scalar2=None,
                                    op0=mybir.AluOpType.mult)
                        # last chunk: per-co DMAs on alternating queues so
                        # the post-matmul tail is one small empty-pipe
                        # transfer, not a backlog of big ones
                        step = (1 if co >= 12 else 2) if last else 4
                        if co % step == step - 1:
                            j0 = co % 4 - (step - 1)
                            # co14/15 go on sync: its DMA dispatch is half
                            # the scalar queue's, and it's empty by then
                            dq = (nc.sync if co >= 14 else nc.scalar) \
                                if (last and co % 2 == 1) else nc.sync
                            dq.dma_start(
                                out=otr[:, co - step + 1:co + 1,
                                        self.ic * 512:self.ic * 512 + 512],
                                in_=self.ot[:, j0:j0 + step, :])

            def finish(self):
                self.finishing = True
                self.step(len(self.items) - self.pos)

        class Qweave:
            """q-projection of chunk 1, head-tiles mt0-mt1, woven a few
            matmuls at a time into attention(0)'s gaps (no ph3 exists yet,
            and attention(0) is DVE-chain-bound)."""

            N_MT = 2

            def __init__(self, xb1, xb1r):
                self.qcur = qpool.tile([P, H_LOC, 512], F16, tag="qcur",
                                       name="qcur1")
                self.terms = ((wq_t[0], xb1), (wq_t[1], xb1), (wq_t[0], xb1r))
                self.items = [(mt, ti, a) for mt in range(self.N_MT)
                              for ti in range(3) for a in range(NP)]
                self.pos = 0
                self.ps = None

            def step(self, n=1):
                for _ in range(n):
                    if self.pos >= len(self.items):
                        return
                    mt, ti, a = self.items[self.pos]
                    self.pos += 1
                    if ti == 0 and a == 0:
                        self.ps = psA.tile([P, 512], F32, tag="psA",
                                           name=f"psqw{mt}")
                    wt, xt = self.terms[ti]
                    nc.tensor.matmul(
                        self.ps[:],
                        lhsT=wt[:, 2 * a:2 * a + 2, mt * P:(mt + 1) * P],
                        rhs=xt[:, 2 * a:2 * a + 2, :],
                        start=(ti == 0 and a == 0),
                        stop=(ti == 2 and a == NP - 1),
                        perf_mode=mybir.MatmulPerfMode.DoubleRow)
                    if ti == 2 and a == NP - 1:
                        nc.scalar.activation(
                            self.qcur[:, mt, :], self.ps[:],
                            mybir.ActivationFunctionType.Identity,
                            bias=bq_sb[:, mt:mt + 1], scale=INV_S)
                        rope_inplace(self.qcur[:, mt, :],
                                     self.qcur[:, mt, :], 512)

        pending = None
        qw = None

        for ic in range(NT):
            ts0 = ic * 512
            x_cur = (xb0, xb0r) if ic == 0 else x_next

            # ---- proj q ----
            _mark(f"pre_q{ic}")
            pre_ex = {}
            if ic == 1 and qw is not None:
                qcur = qw.qcur    # first tiles computed during attention(0)
                mt_start = qw.N_MT
            else:
                qcur = qpool.tile([P, H_LOC, 512], F16, tag="qcur")
                mt_start = 0
            xb8, xb8r = x_cur
            q_terms = fb_terms(wq_t, xb8, xb8r)
            if ic == 0:
                # term-major with 4 concurrent PSUM groups: the main
                # wq8*x8 term starts as soon as the first x8/wq8 DMA chunks
                # land; residual streams arrive while it runs
                ps_q = [psA.tile([P, 512], F32, tag="psA", name=f"psq{m}")
                        for m in range(2)]
                ps_q += [psV.tile([P, 512], F32, tag="psV", name=f"psq{m}")
                         for m in range(2, 4)]
                q_terms0 = ((wq_t[0], xb8), (wq_t[0], xb8r), (wq_t[1], xb8))
                for ti, (wt, xt) in enumerate(q_terms0):
                    for a in range(NP):
                        for mt in range(H_LOC):
                            nc.tensor.matmul(
                                ps_q[mt][:],
                                lhsT=wt[:, 2 * a:2 * a + 2,
                                        mt * P:(mt + 1) * P],
                                rhs=xt[:, 2 * a:2 * a + 2, :],
                                start=(a == 0 and ti == 0),
                                stop=(a == NP - 1 and ti == 2),
                                perf_mode=mybir.MatmulPerfMode.DoubleRow)
                for mt in range(H_LOC):
                    with nc.allow_low_precision(reason="fp16 q"):
                        nc.vector.tensor_scalar(
                            out=qcur[:, mt, :], in0=ps_q[mt][:],
                            scalar1=INV_S, scalar2=bq_sb[:, mt:mt + 1],
                            op0=mybir.AluOpType.mult,
                            op1=mybir.AluOpType.add)
                    rope_inplace(qcur[:, mt, :], qcur[:, mt, :], ts0,
                                 dq=nc.scalar)
            else:
                for mt in range(mt_start, H_LOC):
                    ps = psA.tile([P, 512], F32, tag="psA")
                    for ti, (wt, xt) in enumerate(q_terms):
                        for a in range(NP):
                            nc.tensor.matmul(
                                ps[:],
                                lhsT=wt[:, 2 * a:2 * a + 2,
                                        mt * P:(mt + 1) * P],
                                rhs=xt[:, 2 * a:2 * a + 2, :],
                                start=(a == 0 and ti == 0),
                                stop=(a == NP - 1 and ti == 2),
                                perf_mode=mybir.MatmulPerfMode.DoubleRow)
                    nc.scalar.activation(
                        qcur[:, mt, :], ps[:],
                        mybir.ActivationFunctionType.Identity,
                        bias=bq_sb[:, mt:mt + 1], scale=INV_S)
                    rope_inplace(qcur[:, mt, :], qcur[:, mt, :], ts0)

            def proj_k():
                if ic == 0:
                    # w-residual term last: wk8r chunks are the last arrivals
                    k_terms = ((wk_t[0], xb8), (wk_t[0], xb8r), (wk_t[1], xb8))
                else:
                    k_terms = fb_terms(wk_t, xb8, xb8r)
                for mt in range(H_LOC):
                    ps = psA.tile([P, 512], F32, tag="psA")
                    for ti, (wt, xt) in enumerate(k_terms):
                        for a in range(NP):
                            nc.tensor.matmul(
                                ps[:],
                                lhsT=wt[:, 2 * a:2 * a + 2,
                                        mt * P:(mt + 1) * P],
                                rhs=xt[:, 2 * a:2 * a + 2, :],
                                start=(a == 0 and ti == 0),
                                stop=(a == NP - 1 and ti == 2),
                                perf_mode=mybir.MatmulPerfMode.DoubleRow)
                    if ic == 0:
                        with nc.allow_low_precision(reason="fp16 k"):
                            nc.vector.tensor_scalar(
                                out=k_res[:, mt, ts0:ts0 + 512], in0=ps[:],
                                scalar1=INV_S, scalar2=bk_sb[:, mt:mt + 1],
                                op0=mybir.AluOpType.mult,
                                op1=mybir.AluOpType.add)
                    else:
                        nc.scalar.activation(
                            k_res[:, mt, ts0:ts0 + 512], ps[:],
                            mybir.ActivationFunctionType.Identity,
                            bias=bk_sb[:, mt:mt + 1], scale=INV_S)
                    rope_inplace(k_res[:, mt, ts0:ts0 + 512],
                                 k_res[:, mt, ts0:ts0 + 512], ts0,
                                 dq=nc.scalar if ic == 0 else None)

            def proj_v():
                v_terms = ((xb8, wv_t[0]), (xb8, wv_t[1]), (xb8r, wv_t[0]))
                if ic == 0:
                    # pair-major on psS (idle before attention); term-major
                    # with the w-residual term last, since the residual
                    # weight chunks are the last DMAs to land
                    ps_v = [psA.tile([P, M], F32, tag="psA", name=f"psv{t}")
                            for t in range(2)]
                    ps_v += [psV.tile([P, M], F32, tag="psV", name=f"psv{t}")
                             for t in range(2, 4)]
                    vt0 = ((xb8, wv_t[0]), (xb8r, wv_t[0]), (xb8, wv_t[1]))
                    for ti, (xt, wt) in enumerate(vt0):
                        for a in range(NP):
                            for tt in range(4):
                                nc.tensor.matmul(
                                    ps_v[tt][:],
                                    lhsT=xt[:, 2 * a:2 * a + 2,
                                            tt * P:(tt + 1) * P],
                                    rhs=wt[:, 2 * a:2 * a + 2, :],
                                    start=(a == 0 and ti == 0),
                                    stop=(a == NP - 1 and ti == 2),
                                    perf_mode=mybir.MatmulPerfMode.DoubleRow)
                    for tt in range(4):
                        nc.vector.scalar_tensor_tensor(
                            out=v_res[:, 4 * ic + tt, :], in0=ps_v[tt][:],
                            scalar=INV_S, in1=bvb_sb[:],
                            op0=mybir.AluOpType.mult, op1=mybir.AluOpType.add)
                    return
                for tt in range(4):
                    ps = psA.tile([P, M], F32, tag="psA")
                    for ti, (xt, wt) in enumerate(v_terms):
                        for a in range(NP):
                            nc.tensor.matmul(
                                ps[:],
                                lhsT=xt[:, 2 * a:2 * a + 2,
                                        tt * P:(tt + 1) * P],
                                rhs=wt[:, 2 * a:2 * a + 2, :],
                                start=(a == 0 and ti == 0),
                                stop=(a == NP - 1 and ti == 2),
                                perf_mode=mybir.MatmulPerfMode.DoubleRow)
                    nc.vector.scalar_tensor_tensor(
                        out=v_res[:, 4 * ic + tt, :], in0=ps[:],
                        scalar=INV_S, in1=bvb_sb[:],
                        op0=mybir.AluOpType.mult, op1=mybir.AluOpType.add)

            if ic == 0:
                # wk lands last on the SP queue: fill the gap with proj v
                proj_v()
                _mark(f"v{ic}")
                proj_k()
                _mark(f"k{ic}")
            else:
                proj_k()
                _mark(f"k{ic}")
                if ic == NT - 1:
                    # att3 is Act-exp-capacity-bound while the projections
                    # are Act-idle: pull head-0's first off-diag score pairs
                    # forward so their exps overlap the v projection
                    for pj in range(4):
                        jt0, jt1 = 2 * pj, 2 * pj + 1
                        ps2p = psS.tile([P, 2, 512], F32, tag="psS2")
                        nc.tensor.matmul(
                            ps2p[:, 0, :],
                            lhsT=k_res[:, 0, jt0 * P:(jt0 + 1) * P],
                            rhs=qcur[:, 0, :], start=True, stop=True)
                        nc.tensor.matmul(
                            ps2p[:, 1, :],
                            lhsT=k_res[:, 0, jt1 * P:(jt1 + 1) * P],
                            rhs=qcur[:, 0, :], start=True, stop=True)
                        ex2p = expool.tile([P, 2, 512], F16, tag="ex")
                        nc.scalar.activation(
                            ex2p[:, :, :], ps2p[:, :, :],
                            mybir.ActivationFunctionType.Exp,
                            bias=eb_sb[:, 0:1], scale=SCALE)
                        pre_ex[pj] = ex2p
                proj_v()
                _mark(f"v{ic}")

            if ic + 1 < NT:
                x_next = load_x(ic + 1)

            if ic == 0:
                # wo is first needed by ph3(0) woven into attention(1);
                # dispatch after x1 so it doesn't steal DMA bandwidth from
                # the wv/wk/x1 loads the pipeline stalls on. Sync queue: the
                # scalar queue is head-of-line blocked on chunk-0 rope
                # shuffles; the gpsimd SWDGE path trickles.
                for j in range(H_LOC):
                    nc.sync.dma_start(out=wo_b[:, j:j + 1, :],
                                      in_=wor[:, j:j + 1, :])
                    nc.sync.dma_start(out=wor_b[:, j:j + 1, :],
                                      in_=worr[:, j:j + 1, :])

            # ---- attention for chunk ic (weaving ph3 of chunk ic-1) ----
            attn8 = atpool.tile([P, H_LOC, 512], F8, tag="attn8")
            attn8r = atpool.tile([P, H_LOC, 512], F8, tag="attn8r")
            njt = 4 * ic + 4
            slots_left = H_LOC * njt

            def emit_chain(dfr):
                """Deferred per-head softmax-normalize chain: recip + fp8
                attn split. Deferred into the NEXT head's tile loop so the
                in-order DVE queue never parks waiting for the Pool
                all_reduce (which sits behind woven ph3 evicts)."""
                rden_d, ps_av_d, hh = dfr
                with nc.allow_low_precision(reason="softmax reciprocal"):
                    if rden_d.space == tile.bass.MemorySpace.PSUM:
                        den_ps = rden_d
                        rden_d = rdpool.tile([P, 512], F16, tag="rden")
                        nc.vector.reciprocal(rden_d[:], den_ps[:, 0, :])
                    else:
                        nc.vector.reciprocal(rden_d[:], rden_d[:])
                    nc.vector.scalar_tensor_tensor(
                        out=attn8[:, hh, :], in0=ps_av_d[:], scalar=SA,
                        in1=rden_d[:],
                        op0=mybir.AluOpType.mult, op1=mybir.AluOpType.mult)
                    a16 = a16pool.tile([P, 512], F16, tag="a16")
                    nc.vector.scalar_tensor_tensor(
                        out=a16[:], in0=ps_av_d[:], scalar=SA, in1=rden_d[:],
                        op0=mybir.AluOpType.mult, op1=mybir.AluOpType.mult)
                    eng = nc.vector if hh == H_LOC - 1 else nc.gpsimd
                    eng.tensor_tensor(
                        attn8r[:, hh, :], a16[:], attn8[:, hh, :],
                        mybir.AluOpType.subtract)

            deferred = None
            ic0_chains = []
            n_items = CT * 6
            slots_done = 0
            for h in range(H_LOC):
                ps_av = psV.tile([P, 512], F32, tag="psV")
                exsum = espool.tile([P, 512], F16, tag="exsum")
                prev = None  # (ex2 tile, c00, c01, jt0) awaiting av matmuls
                for pj in range(njt // 2):
                    jt0, jt1 = 2 * pj, 2 * pj + 1
                    d0, d1 = jt0 - 4 * ic, jt1 - 4 * ic
                    c00 = 128 * d0 if d0 > 0 else 0
                    c01 = 128 * d1 if d1 > 0 else 0
                    if pj == min(2, njt // 2 - 1) and deferred is not None:
                        emit_chain(deferred)
                        deferred = None
                    if h == 0 and pj in pre_ex:
                        # precomputed during the projection phase
                        ex2 = pre_ex[pj]
                        slots_done += 2
                        if pending is not None:
                            eff = slots_left + (36 if ic == NT - 1 else 0)
                            tgt = min(n_items,
                                      (n_items * slots_done) // eff)
                            pending.step(tgt - pending.pos)
                        if prev is not None:
                            pex2, pc00, pc01, pjt0 = prev
                            nc.tensor.matmul(
                                ps_av[:, pc00:],
                                lhsT=v_res[:, pjt0, h * HS:(h + 1) * HS],
                                rhs=pex2[:, 0, pc00:],
                                start=(pjt0 == 0), stop=False,
                                skip_group_check=True)
                            nc.tensor.matmul(
                                ps_av[:, pc01:],
                                lhsT=v_res[:, pjt0 + 1, h * HS:(h + 1) * HS],
                                rhs=pex2[:, 1, pc01:],
                                start=False, stop=False,
                                skip_group_check=True)
                        with nc.allow_low_precision(reason="fp16 denom"):
                            if pj == 0:
                                nc.vector.tensor_tensor(
                                    exsum[:], ex2[:, 0, :], ex2[:, 1, :],
                                    mybir.AluOpType.add)
                            else:
                                nc.vector.tensor_tensor(
                                    exsum[:], exsum[:],
                                    ex2[:, 0, :], mybir.AluOpType.add)
                                nc.vector.tensor_tensor(
                                    exsum[:], exsum[:],
                                    ex2[:, 1, :], mybir.AluOpType.add)
                        prev = (ex2, 0, 0, jt0)
                        continue
                    # scores for a PAIR of key tiles into one 2-bank PSUM
                    # tile; one Act exp over both planes amortizes the
                    # fixed per-op Act overhead (the attention pacer)
                    ps2 = psS.tile([P, 2, 512], F32, tag="psS2")
                    nc.tensor.matmul(
                        ps2[:, 0, c00:],
                        lhsT=k_res[:, h, jt0 * P:(jt0 + 1) * P],
                        rhs=qcur[:, h, c00:],
                        start=True, stop=True)
                    split = d0 >= 0 and ic < NT - 1
                    # diagonal pairs in PE-bound windows: exact-trim plane 1
                    # and pay a second (smaller) exp on the slack Act engine.
                    # Otherwise plane 1 also starts at c00: cols [c00:c01]
                    # are real (causal-masked) scores so the shared exp
                    # never reads uninitialized PSUM
                    c1lo = c01 if split else c00
                    nc.tensor.matmul(
                        ps2[:, 1, c1lo:],
                        lhsT=k_res[:, h, jt1 * P:(jt1 + 1) * P],
                        rhs=qcur[:, h, c1lo:],
                        start=True, stop=True)
                    ex2 = expool.tile([P, 2, 512], F16, tag="ex")
                    if split:
                        nc.scalar.activation(
                            ex2[:, 0, c00:], ps2[:, 0, c00:],
                            mybir.ActivationFunctionType.Exp,
                            bias=eb_sb[:, 0:1], scale=SCALE)
                        nc.scalar.activation(
                            ex2[:, 1, c01:], ps2[:, 1, c01:],
                            mybir.ActivationFunctionType.Exp,
                            bias=eb_sb[:, 0:1], scale=SCALE)
                    else:
                        nc.scalar.activation(
                            ex2[:, :, c00:], ps2[:, :, c00:],
                            mybir.ActivationFunctionType.Exp,
                            bias=eb_sb[:, 0:1], scale=SCALE)
                    # causal triangle: for every diagonal tile the global
                    # query base ts0+c0 equals the key base jt*P, so one
                    # [P,P] keep-where-col>=row mask serves them all
                    if d0 >= 0:
                        nc.vector.tensor_tensor(
                            ex2[:, 0, c00:c00 + P], ex2[:, 0, c00:c00 + P],
                            tri_sb[:], mybir.AluOpType.mult)
                    if d1 >= 0:
                        nc.vector.tensor_tensor(
                            ex2[:, 1, c01:c01 + P], ex2[:, 1, c01:c01 + P],
                            tri_sb[:], mybir.AluOpType.mult)
                    slots_done += 2
                    if pending is not None:
                        # spread ph3 items evenly across the window; on the
                        # last chunk hold ~16 back so they bridge the final
                        # head's normalize-chain latency after the window
                        eff = slots_left + (36 if ic == NT - 1 else 0)
                        tgt = min(n_items, (n_items * slots_done) // eff)
                        pending.step(tgt - pending.pos)
                    elif qw is not None:
                        nqw = len(qw.items)
                        qw.step((nqw * slots_done) // slots_left - qw.pos)
                    if prev is not None:
                        pex2, pc00, pc01, pjt0 = prev
                        nc.tensor.matmul(
                            ps_av[:, pc00:],
                            lhsT=v_res[:, pjt0, h * HS:(h + 1) * HS],
                            rhs=pex2[:, 0, pc00:],
                            start=(pjt0 == 0), stop=False,
                            skip_group_check=True)
                        nc.tensor.matmul(
                            ps_av[:, pc01:],
                            lhsT=v_res[:, pjt0 + 1, h * HS:(h + 1) * HS],
                            rhs=pex2[:, 1, pc01:],
                            start=False, stop=False,
                            skip_group_check=True)
                    with nc.allow_low_precision(reason="fp16 softmax denom"):
                        if pj == 0:
                            if ic == 0:
                                nc.vector.tensor_copy(out=exsum[:],
                                                      in_=ex2[:, 0, :])
                                nc.vector.tensor_tensor(
                                    exsum[:, c01:], exsum[:, c01:],
                                    ex2[:, 1, c01:], mybir.AluOpType.add)
                            else:
                                nc.vector.tensor_tensor(
                                    exsum[:], ex2[:, 0, :], ex2[:, 1, :],
                                    mybir.AluOpType.add)
                        else:
                            nc.vector.tensor_tensor(
                                exsum[:, c00:], exsum[:, c00:],
                                ex2[:, 0, c00:], mybir.AluOpType.add)
                            nc.vector.tensor_tensor(
                                exsum[:, c01:], exsum[:, c01:],
                                ex2[:, 1, c01:], mybir.AluOpType.add)
                    prev = (ex2, c00, c01, jt0)
                pex2, pc00, pc01, pjt0 = prev
                nc.tensor.matmul(
                    ps_av[:, pc00:],
                    lhsT=v_res[:, pjt0, h * HS:(h + 1) * HS],
                    rhs=pex2[:, 0, pc00:],
                    start=(pjt0 == 0), stop=False,
                    skip_group_check=True)
                nc.tensor.matmul(
                    ps_av[:, pc01:],
                    lhsT=v_res[:, pjt0 + 1, h * HS:(h + 1) * HS],
                    rhs=pex2[:, 1, pc01:],
                    start=False, stop=True,
                    skip_group_check=True)
                if (h == H_LOC - 1 and ic == NT - 1) or ic == 0:
                    # chunk 0 (PE idle, DVE-bound) and end of kernel (PE
                    # free, chain gates the output projection): sum the
                    # denominator with a ones-matmul instead of the slower
                    # Pool all_reduce
                    ps_d = psS.tile([P, 2, 512], F32, tag="psS2")
                    nc.tensor.matmul(
                        ps_d[:, 0, :], lhsT=ones_sb[:], rhs=exsum[:],
                        start=True, stop=True)
                    deferred = (ps_d, ps_av, h)
                else:
                    rden = rdpool.tile([P, 512], F16, tag="rden")
                    nc.gpsimd.partition_all_reduce(
                        rden[:], exsum[:], channels=P,
                        reduce_op=bass_isa.ReduceOp.add)
                    deferred = (rden, ps_av, h)
                if h == H_LOC - 1 and ic < NT - 1:
                    emit_chain(deferred)
                    deferred = None

            _mark(f"att{ic}")
            for dfr in ic0_chains:
                emit_chain(dfr)
            if pending is not None:
                pending.finish()
            _mark(f"ph3fin{ic}")
            pending = Ph3(ic, attn8, attn8r,
                          pools=[psA, psV] if ic == NT - 1 else None)
            if deferred is not None:
                # last chunk: co0's three g0 steps only read heads 0-1; emit
                # them BEFORE the last head's normalize chain so they don't
                # inherit a wait on its attn8 write (sem counts are
                # emission-order conservative) and bridge its latency
                pending.step(6)
                emit_chain(deferred)
                deferred = None

        pending.finish()
        _mark("ph3last")

    nc.finalize()
    return nc


def get_nc(phases=(1, 2, 3)):
    if phases not in _NC_CACHE:
        _NC_CACHE[phases] = _build(phases)
    return _NC_CACHE[phases]


def _rope_tables():
    inv_freq = 1.0 / (BASE ** (np.arange(0, ROT, 2, dtype=np.float64) / ROT))
    freqs = np.arange(T, dtype=np.float64)[:, None] * inv_freq[None, :]  # [T, 32]
    cos_h = np.cos(freqs).T.astype(np.float32)   # [32, T]
    sin_h = np.sin(freqs).T.astype(np.float32)
    cosT = np.concatenate([cos_h, cos_h], axis=0)          # [64, T]
    nsT = np.concatenate([-sin_h, sin_h], axis=0)          # [64, T] signed sin
    return (np.ascontiguousarray(cosT.astype(np.float16)),
            np.ascontiguousarray(nsT.astype(np.float16)))


def _split8(a, s):
    """a*s = a8 + a8r (both fp8 e4m3) up to second-order quantization."""
    scaled = a * np.float32(s)
    a8 = scaled.astype(NPF8)
    a8r = (scaled - a8.astype(np.float32)).astype(NPF8)
    return np.ascontiguousarray(a8), np.ascontiguousarray(a8r)


def make_in_maps(x, Wq, bq, Wk, bk, Wv, bv, Wo, bo):
    cosT, nsT = _rope_tables()
    in_maps = []
    for c in range(N_CORES):
        b, g = divmod(c, TPG)
        ms = slice(g * M, (g + 1) * M)
        xh8, xh8r = _split8(x[b].T, SX)
        wq8, wq8r = _split8(Wq[ms].T, SW)
        wk8, wk8r = _split8(Wk[ms].T, SW)
        wv8, wv8r = _split8(Wv[ms].T, SW)
        wo8, wo8r = _split8(Wo[:, ms].T, SWO)
        in_maps.append({
            "xh8": xh8, "xh8r": xh8r,
            "wq8": wq8, "wq8r": wq8r,
            "wk8": wk8, "wk8r": wk8r,
            "wv8": wv8, "wv8r": wv8r,
            "wo8T": wo8, "wo8rT": wo8r,
            "bqc": np.ascontiguousarray(bq[ms].reshape(H_LOC, P).T),
            "bkc": np.ascontiguousarray(bk[ms].reshape(H_LOC, P).T),
            "bvr": np.ascontiguousarray(bv[ms].reshape(1, M).astype(np.float16)),
            "ebias": np.full((P, 1), EXP_BIAS, np.float32),
            "trimask": np.triu(np.ones((P, P), np.float16)),
            "cosT": cosT,
            "nsT": nsT,
        })
    return in_maps


def assemble(results, bo):
    out = np.empty((B, T, C), dtype=np.float32)
    for b in range(B):
        acc = results[b * TPG]["outT"].astype(np.float32)
        for g in range(1, TPG):
            acc = acc + results[b * TPG + g]["outT"].astype(np.float32)
        out[b] = acc.T + bo[None, :]
    return out


def kernel(x, Wq, bq, Wk, bk, Wv, bv, Wo, bo):
    nc = get_nc()
    in_maps = make_in_maps(np.asarray(x, np.float32),
                           np.asarray(Wq, np.float32), np.asarray(bq, np.float32),
                           np.asarray(Wk, np.float32), np.asarray(bk, np.float32),
                           np.asarray(Wv, np.float32), np.asarray(bv, np.float32),
                           np.asarray(Wo, np.float32), np.asarray(bo, np.float32))
    res = run_bass_kernel_spmd(nc, in_maps, list(range(N_CORES)))
    return assemble(res.results, np.asarray(bo, np.float32))



# revision 92
# speedup vs baseline: 1.0008x; 1.0008x over previous
"""Trainium2 Bass kernel for a full causal MHA layer (B=2, T=2048, C=2048, H=16,
partial RoPE on first 64 dims of each 128-dim head).

Sharding over 8 cores: core c handles batch b=c//4 and heads [4g, 4g+4), g=c%4
(tensor-parallel over heads x data-parallel over batch).

Fully fused single pass per core, fp16 data plane (fp32 PSUM accumulation):
  for each 512-token chunk ic:
    proj q/k (fp16 weights stationary, fp16 x moving), bias+partial-RoPE,
      q and k stay resident in SBUF (no DRAM spills)
    proj v -> v_res [key, jt, m] fp16 resident
    attention for chunk ic over heads h: per key-tile jt
      scoresT[k,q] (k_res stationary fp16, q moving fp16)
      -> exp(scale*s - 10*ln2) -> ex fp16 (Act), triangle mask on diagonal
         tiles only (DVE mult by a const [128,128] triu mask), exact causal
         col-trimming
      -> av accumulation outT[d,q] via PE; softmax denominator via DVE
         exsum adds + gpsimd partition_all_reduce (no PE ones-matmuls)
      output-projection matmuls of chunk ic-1 are woven between attention
      matmuls to keep PE busy during Act-latency windows
    phase3(ic): out partial outT[c,q] = sum_mt woT attn, DVE evict fp16, DMA
Host: slices inputs per core (fp16), sums the 4 TP partials per batch + bo.
"""

import math

import ml_dtypes
import numpy as np

NPF8 = ml_dtypes.float8_e4m3

import concourse.bass_isa as bass_isa
import concourse.mybir as mybir
import concourse.tile as tile
from concourse import bacc
from concourse.bass_utils import run_bass_kernel_spmd

F32 = mybir.dt.float32
F16 = mybir.dt.float16
F8 = mybir.dt.float8e4

B, T, C = 2, 2048, 2048
H = 16
HS = 128
ROT = 64
HALF = 32
BASE = 10000.0

N_CORES = 8
TPG = 4                # TP group size (heads split)
H_LOC = H // TPG       # 4 heads per core
M = H_LOC * HS         # 512 local head-dim columns
SCALE = 1.0 / math.sqrt(HS)
EXP_BIAS = -10.0 * math.log(2.0)   # exp(s*SCALE - 10ln2): keeps fp16 in range
SX = 16.0                 # fp8 quantization scale for x
SW = 1024.0               # fp8 quantization scale for Wq/Wk/Wv
INV_S = 1.0 / (SX * SW)   # folded into the projection evictions
SWO = 1024.0              # fp8 quantization scale for Wo
SA = 64.0                 # fp8 quantization scale for attn outputs
INV_SO = 1.0 / (SWO * SA)  # folded into the output-proj evictions

P = 128
NT = T // 512          # 4 t-chunks of 512
CT = C // P            # 16 contraction tiles
JT = T // P            # 16 key tiles per head

_NC_CACHE = {}
PHASE_MARKS = []  # (label, last-emitted instruction name); debug aid only


def _build(phases=(1, 2, 3)):
    nc = bacc.Bacc(None, target_bir_lowering=False)
    PHASE_MARKS.clear()

    def _mark(label):
        # consumes one instruction name as a monotonic position marker
        PHASE_MARKS.append((label, nc.get_next_instruction_name()))

    xh8 = nc.declare_dram_parameter("xh8", [C, T], F8, isOutput=False)
    xh8r = nc.declare_dram_parameter("xh8r", [C, T], F8, isOutput=False)
    wq8 = nc.declare_dram_parameter("wq8", [C, M], F8, isOutput=False)
    wq8r = nc.declare_dram_parameter("wq8r", [C, M], F8, isOutput=False)
    wk8 = nc.declare_dram_parameter("wk8", [C, M], F8, isOutput=False)
    wk8r = nc.declare_dram_parameter("wk8r", [C, M], F8, isOutput=False)
    wv8 = nc.declare_dram_parameter("wv8", [C, M], F8, isOutput=False)
    wv8r = nc.declare_dram_parameter("wv8r", [C, M], F8, isOutput=False)
    wo8T = nc.declare_dram_parameter("wo8T", [M, C], F8, isOutput=False)
    wo8rT = nc.declare_dram_parameter("wo8rT", [M, C], F8, isOutput=False)
    bqc = nc.declare_dram_parameter("bqc", [P, H_LOC], F32, isOutput=False)
    bkc = nc.declare_dram_parameter("bkc", [P, H_LOC], F32, isOutput=False)
    bvr = nc.declare_dram_parameter("bvr", [1, M], F16, isOutput=False)
    ebias = nc.declare_dram_parameter("ebias", [P, 1], F32, isOutput=False)
    trimask = nc.declare_dram_parameter("trimask", [P, P], F16, isOutput=False)
    cosT = nc.declare_dram_parameter("cosT", [ROT, T], F16, isOutput=False)
    nsT = nc.declare_dram_parameter("nsT", [ROT, T], F16, isOutput=False)
    outT = nc.declare_dram_parameter("outT", [C, T], F16, isOutput=True)

    with tile.TileContext(nc) as tc, \
         tc.tile_pool(name="const", bufs=1) as const, \
         tc.tile_pool(name="xp", bufs=CT) as xpool, \
         tc.tile_pool(name="qc", bufs=2) as qpool, \
         tc.tile_pool(name="at", bufs=2) as atpool, \
         tc.tile_pool(name="rp", bufs=6) as rpool, \
         tc.tile_pool(name="exp", bufs=5) as expool, \
         tc.tile_pool(name="exs", bufs=3) as espool, \
         tc.tile_pool(name="rd", bufs=5) as rdpool, \
         tc.tile_pool(name="a16", bufs=2) as a16pool, \
         tc.tile_pool(name="oe", bufs=3) as oepool, \
         tc.tile_pool(name="psA", bufs=2, space="PSUM") as psA, \
         tc.tile_pool(name="psS", bufs=2, space="PSUM") as psS, \
         tc.tile_pool(name="psV", bufs=2, space="PSUM") as psV:

        cos_sb = const.tile([ROT, T], F16, tag="cos")
        ns_sb = const.tile([ROT, T], F16, tag="ns")
        bq_sb = const.tile([P, H_LOC], F32, tag="bq")
        bk_sb = const.tile([P, H_LOC], F32, tag="bk")
        bvb_sb = const.tile([P, M], F16, tag="bvb")
        eb_sb = const.tile([P, 1], F32, tag="ebias")
        tri_sb = const.tile([P, P], F16, tag="trimask")
        k_res = const.tile([P, H_LOC, T], F16, tag="kres")
        v_res = const.tile([P, JT, M], F16, tag="vres")
        wq_t = [const.tile([P, CT, M], F8, tag=f"wq{i}", name=f"wq{i}")
                for i in range(2)]
        wk_t = [const.tile([P, CT, M], F8, tag=f"wk{i}", name=f"wk{i}")
                for i in range(2)]
        wv_t = [const.tile([P, CT, M], F8, tag=f"wv{i}", name=f"wv{i}")
                for i in range(2)]
        wo_b = const.tile([P, H_LOC, C], F8, tag="wob")
        wor_b = const.tile([P, H_LOC, C], F8, tag="worb")
        wre = [d[:].rearrange("(ct p) m -> p ct m", p=P)
               for d in (wq8, wq8r, wk8, wk8r, wv8, wv8r)]
        wqre, wqrre, wkre, wkrre, wvre, wvrre = wre
        wor = wo8T[:].rearrange("(mt p) c -> p mt c", p=P)
        worr = wo8rT[:].rearrange("(mt p) c -> p mt c", p=P)

        xr = xh8[:].rearrange("(ct p) t -> p ct t", p=P)
        xrr = xh8r[:].rearrange("(ct p) t -> p ct t", p=P)
        otr = outT[:].rearrange("(co p) t -> p co t", p=P)

        def load_x(ic):
            ts0 = ic * 512
            xb = xpool.tile([P, CT, 512], F8, tag="xb8", name=f"xb{ic}", bufs=2)
            xbr = xpool.tile([P, CT, 512], F8, tag="xb8r", name=f"xbr{ic}",
                             bufs=2)
            for j in range(2):
                nc.sync.dma_start(out=xb[:, 8 * j:8 * j + 8, :],
                                  in_=xr[:, 8 * j:8 * j + 8, ts0:ts0 + 512])
                nc.sync.dma_start(out=xbr[:, 8 * j:8 * j + 8, :],
                                  in_=xrr[:, 8 * j:8 * j + 8, ts0:ts0 + 512])
            return xb, xbr

        # startup, ordered by first use: main-term operands (x8 + wq8) first,
        # then the residual streams, then wv (main, res), wk (main, res).
        # x on the SP queue, wq on the Act queue so the two interleave on the
        # DMA engines; chunked so HWDGE overheads don't pace it.
        xb0 = xpool.tile([P, CT, 512], F8, tag="xb8", name="xb0", bufs=2)
        xb0r = xpool.tile([P, CT, 512], F8, tag="xb8r", name="xb0r", bufs=2)
        # first ct-pair as its own small DMA so the very first matmul can
        # start as early as possible
        nc.sync.dma_start(out=xb0[:, 0:2, :], in_=xr[:, 0:2, 0:512])
        nc.scalar.dma_start(out=wq_t[0][:, 0:2, :], in_=wqre[:, 0:2, :])
        nc.sync.dma_start(out=xb0[:, 2:4, :], in_=xr[:, 2:4, 0:512])
        nc.scalar.dma_start(out=wq_t[0][:, 2:4, :], in_=wqre[:, 2:4, :])
        for j in range(1, 4):
            a, b = 4 * j, 4 * j + 4
            nc.sync.dma_start(out=xb0[:, a:b, :], in_=xr[:, a:b, 0:512])
            nc.scalar.dma_start(out=wq_t[0][:, a:b, :], in_=wqre[:, a:b, :])
        for j in range(4):
            a, b = 4 * j, 4 * j + 4
            nc.sync.dma_start(out=xb0r[:, a:b, :], in_=xrr[:, a:b, 0:512])
            nc.scalar.dma_start(out=wq_t[1][:, a:b, :], in_=wqrre[:, a:b, :])
        for j in range(4):
            a, b = 4 * j, 4 * j + 4
            nc.sync.dma_start(out=wv_t[0][:, a:b, :], in_=wvre[:, a:b, :])
        for j in range(4):
            a, b = 4 * j, 4 * j + 4
            nc.sync.dma_start(out=wv_t[1][:, a:b, :], in_=wvrre[:, a:b, :])
        for j in range(4):
            a, b = 4 * j, 4 * j + 4
            nc.sync.dma_start(out=wk_t[0][:, a:b, :], in_=wkre[:, a:b, :])
        for j in range(4):
            a, b = 4 * j, 4 * j + 4
            nc.sync.dma_start(out=wk_t[1][:, a:b, :], in_=wkrre[:, a:b, :])
        ones_sb = const.tile([P, P], F16, tag="ones")
        nc.gpsimd.memset(ones_sb[:], 1.0)
        nc.gpsimd.dma_start(out=bq_sb[:], in_=bqc[:])
        nc.gpsimd.dma_start(out=bk_sb[:], in_=bkc[:])
        nc.gpsimd.dma_start(out=cos_sb[:], in_=cosT[:])
        nc.gpsimd.dma_start(out=ns_sb[:], in_=nsT[:])
        nc.gpsimd.dma_start(out=eb_sb[:], in_=ebias[:])
        nc.gpsimd.dma_start(out=tri_sb[:], in_=trimask[:])
        nc.gpsimd.dma_start(out=bvb_sb[:], in_=bvr[0:1, :].to_broadcast([P, M]))

        NP = CT // 2   # 8 ct-pairs per contraction

        def fb_terms(w_t, xb, xbr):
            """(lhsT_tile, rhs_tile) per error-feedback term: main, w-res,
            x-res. All DoubleRow fp8 over ct-pairs."""
            return ((w_t[0], xb), (w_t[1], xb), (w_t[0], xbr))

        def rope_inplace(dst, tmp_src, ts0, dq=None):
            """dst[0:ROT, 512] fp16 <- rope(tmp_src rows 0:ROT) in place.
            tmp_src rows are pre-rope biased values; dst may alias tmp_src.
            dq picks the DMA queue for the partition-swap (the SP queue is
            backed up with weight loads during chunk 0)."""
            dq = dq or nc.sync
            sh = rpool.tile([ROT, 512], F16, tag="sh")
            dq.dma_start(out=sh[0:HALF], in_=tmp_src[HALF:ROT])
            dq.dma_start(out=sh[HALF:ROT], in_=tmp_src[0:HALF])
            rot = rpool.tile([ROT, 512], F16, tag="rot")
            nc.vector.tensor_tensor(rot[:], sh[:], ns_sb[:, ts0:ts0 + 512],
                                    mybir.AluOpType.mult)
            tcos = rpool.tile([ROT, 512], F16, tag="tcos")
            nc.vector.tensor_tensor(tcos[:], tmp_src[:ROT], cos_sb[:, ts0:ts0 + 512],
                                    mybir.AluOpType.mult)
            nc.vector.tensor_tensor(dst[0:ROT], tcos[:], rot[:],
                                    mybir.AluOpType.add)

        class Ph3:
            """Output projection for chunk ic (fp8 DoubleRow, 3-term error
            feedback); matmuls are dispensed one at a time (step) so they
            weave between attention matmuls. 6 steps per co:
            (term, g) with term in {wo8*a8, wo8r*a8, wo8*a8r}, g the DR pair
            of mt planes."""

            def __init__(self, ic, a8, a8r, pools=None):
                self.ic = ic
                self.terms = ((wo_b, a8), (wor_b, a8), (wo_b, a8r))
                self.items = [(co, s) for co in range(CT) for s in range(6)]
                if ic == NT - 1:
                    # stagger co0/co1: their g0 steps (heads 0-1 only) first,
                    # g1 steps after — bridges the last head's normalize
                    # chain at the attention/finish boundary
                    self.items = (
                        [(0, 0), (0, 1), (0, 2), (1, 0), (1, 1), (1, 2),
                         (0, 3), (0, 4), (0, 5), (1, 3), (1, 4), (1, 5)]
                        + [(co, s) for co in range(2, CT) for s in range(6)])
                self.pos = 0
                self.ps = None
                self.pools = pools or [psA]
                self.finishing = False

            def step(self, n=1):
                for _ in range(n):
                    if self.pos >= len(self.items):
                        return
                    co, s = self.items[self.pos]
                    self.pos += 1
                    # g-major: the three g=0 steps only read heads 0-1,
                    # which are normalized well before heads 2-3 land
                    g, t = divmod(s, 3)
                    if s == 0:
                        pool = self.pools[co % len(self.pools)]
                        ps = pool.tile([P, 512], F32,
                                       tag="psA" if pool is psA else "psV",
                                       name=f"ps3_{self.ic}_{co}")
                        self.ps_by_co = getattr(self, "ps_by_co", {})
                        self.ps_by_co[co] = ps
                    self.ps = self.ps_by_co[co]
                    wt, at = self.terms[t]
                    nc.tensor.matmul(
                        self.ps[:],
                        lhsT=wt[:, 2 * g:2 * g + 2, co * P:(co + 1) * P],
                        rhs=at[:, 2 * g:2 * g + 2, :],
                        start=(s == 0), stop=(s == 5),
                        perf_mode=mybir.MatmulPerfMode.DoubleRow)
                    if s == 5:
                        if co % 4 == 0:
                            self.ot = oepool.tile([P, 4, 512], F16, tag="ot")
                        last = self.ic == NT - 1
                        if self.finishing and last and co % 2 == 0:
                            # end of kernel: DVE is idle, alternate with Act
                            # so the final evict drain is 2-wide
                            with nc.allow_low_precision(reason="fp16 out"):
                                nc.vector.tensor_scalar(
                                    out=self.ot[:, co % 4, :], in0=self.ps[:],
                                    scalar1=INV_SO, scalar2=None,
                                    op0=mybir.AluOpType.mult)
                        elif self.finishing:
                            # post-attention block: DVE is draining attention
                            # tail work, evict on Act
                            nc.scalar.activation(
                                self.ot[:, co % 4, :], self.ps[:],
                                mybir.ActivationFunctionType.Identity,
                                scale=INV_SO)
                        elif co % 3 == 2:
                            nc.scalar.activation(
                                self.ot[:, co % 4, :], self.ps[:],
                                mybir.ActivationFunctionType.Identity,
                                scale=INV_SO)
                        else:
                            # woven between attention matmuls: mostly DVE
                            # with Act picking up every third (gpsimd can't
                            # read PSUM)
                            with nc.allow_low_precision(reason="fp16 out"):
                                nc.vector.tensor_scalar(
                                    out=self.ot[:, co % 4, :], in0=self.ps[:],
                                    scalar1=INV_SO, scalar2=None,
                                    op0=mybir.AluOpType.mult)
                        # last chunk: per-co DMAs on alternating queues so
                        # the post-matmul tail is one small empty-pipe
                        # transfer, not a backlog of big ones
                        step = (1 if co >= 12 else 2) if last else 2
                        if co % step == step - 1:
                            j0 = co % 4 - (step - 1)
                            # co14/15 go on sync: its DMA dispatch is half
                            # the scalar queue's, and it's empty by then
                            dq = (nc.sync if co >= 14 else nc.scalar) \
                                if (last and co % 2 == 1) else nc.sync
                            dq.dma_start(
                                out=otr[:, co - step + 1:co + 1,
                                        self.ic * 512:self.ic * 512 + 512],
                                in_=self.ot[:, j0:j0 + step, :])

            def finish(self):
                self.finishing = True
                self.step(len(self.items) - self.pos)

        class Qweave:
            """q-projection of chunk 1, head-tiles mt0-mt1, woven a few
            matmuls at a time into attention(0)'s gaps (no ph3 exists yet,
            and attention(0) is DVE-chain-bound)."""

            N_MT = 2

            def __init__(self, xb1, xb1r):
                self.qcur = qpool.tile([P, H_LOC, 512], F16, tag="qcur",
                                       name="qcur1")
                self.terms = ((wq_t[0], xb1), (wq_t[1], xb1), (wq_t[0], xb1r))
                self.items = [(mt, ti, a) for mt in range(self.N_MT)
                              for ti in range(3) for a in range(NP)]
                self.pos = 0
                self.ps = None

            def step(self, n=1):
                for _ in range(n):
                    if self.pos >= len(self.items):
                        return
                    mt, ti, a = self.items[self.pos]
                    self.pos += 1
                    if ti == 0 and a == 0:
                        self.ps = psA.tile([P, 512], F32, tag="psA",
                                           name=f"psqw{mt}")
                    wt, xt = self.terms[ti]
                    nc.tensor.matmul(
                        self.ps[:],
                        lhsT=wt[:, 2 * a:2 * a + 2, mt * P:(mt + 1) * P],
                        rhs=xt[:, 2 * a:2 * a + 2, :],
                        start=(ti == 0 and a == 0),
                        stop=(ti == 2 and a == NP - 1),
                        perf_mode=mybir.MatmulPerfMode.DoubleRow)
                    if ti == 2 and a == NP - 1:
                        nc.scalar.activation(
                            self.qcur[:, mt, :], self.ps[:],
                            mybir.ActivationFunctionType.Identity,
                            bias=bq_sb[:, mt:mt + 1], scale=INV_S)
                        rope_inplace(self.qcur[:, mt, :],
                                     self.qcur[:, mt, :], 512)

        pending = None
        qw = None

        for ic in range(NT):
            ts0 = ic * 512
            x_cur = (xb0, xb0r) if ic == 0 else x_next

            # ---- proj q ----
            _mark(f"pre_q{ic}")
            pre_ex = {}
            if ic == 1 and qw is not None:
                qcur = qw.qcur    # first tiles computed during attention(0)
                mt_start = qw.N_MT
            else:
                qcur = qpool.tile([P, H_LOC, 512], F16, tag="qcur")
                mt_start = 0
            xb8, xb8r = x_cur
            q_terms = fb_terms(wq_t, xb8, xb8r)
            if ic == 0:
                # term-major with 4 concurrent PSUM groups: the main
                # wq8*x8 term starts as soon as the first x8/wq8 DMA chunks
                # land; residual streams arrive while it runs
                ps_q = [psA.tile([P, 512], F32, tag="psA", name=f"psq{m}")
                        for m in range(2)]
                ps_q += [psV.tile([P, 512], F32, tag="psV", name=f"psq{m}")
                         for m in range(2, 4)]
                q_terms0 = ((wq_t[0], xb8), (wq_t[0], xb8r), (wq_t[1], xb8))
                for ti, (wt, xt) in enumerate(q_terms0):
                    for a in range(NP):
                        for mt in range(H_LOC):
                            nc.tensor.matmul(
                                ps_q[mt][:],
                                lhsT=wt[:, 2 * a:2 * a + 2,
                                        mt * P:(mt + 1) * P],
                                rhs=xt[:, 2 * a:2 * a + 2, :],
                                start=(a == 0 and ti == 0),
                                stop=(a == NP - 1 and ti == 2),
                                perf_mode=mybir.MatmulPerfMode.DoubleRow)
                for mt in range(H_LOC):
                    with nc.allow_low_precision(reason="fp16 q"):
                        nc.vector.tensor_scalar(
                            out=qcur[:, mt, :], in0=ps_q[mt][:],
                            scalar1=INV_S, scalar2=bq_sb[:, mt:mt + 1],
                            op0=mybir.AluOpType.mult,
                            op1=mybir.AluOpType.add)
                    rope_inplace(qcur[:, mt, :], qcur[:, mt, :], ts0,
                                 dq=nc.scalar)
            else:
                for mt in range(mt_start, H_LOC):
                    ps = psA.tile([P, 512], F32, tag="psA")
                    for ti, (wt, xt) in enumerate(q_terms):
                        for a in range(NP):
                            nc.tensor.matmul(
                                ps[:],
                                lhsT=wt[:, 2 * a:2 * a + 2,
                                        mt * P:(mt + 1) * P],
                                rhs=xt[:, 2 * a:2 * a + 2, :],
                                start=(a == 0 and ti == 0),
                                stop=(a == NP - 1 and ti == 2),
                                perf_mode=mybir.MatmulPerfMode.DoubleRow)
                    nc.scalar.activation(
                        qcur[:, mt, :], ps[:],
                        mybir.ActivationFunctionType.Identity,
                        bias=bq_sb[:, mt:mt + 1], scale=INV_S)
                    rope_inplace(qcur[:, mt, :], qcur[:, mt, :], ts0)

            def proj_k():
                if ic == 0:
                    # w-residual term last: wk8r chunks are the last arrivals
                    k_terms = ((wk_t[0], xb8), (wk_t[0], xb8r), (wk_t[1], xb8))
                else:
                    k_terms = fb_terms(wk_t, xb8, xb8r)
                for mt in range(H_LOC):
                    ps = psA.tile([P, 512], F32, tag="psA")
                    for ti, (wt, xt) in enumerate(k_terms):
                        for a in range(NP):
                            nc.tensor.matmul(
                                ps[:],
                                lhsT=wt[:, 2 * a:2 * a + 2,
                                        mt * P:(mt + 1) * P],
                                rhs=xt[:, 2 * a:2 * a + 2, :],
                                start=(a == 0 and ti == 0),
                                stop=(a == NP - 1 and ti == 2),
                                perf_mode=mybir.MatmulPerfMode.DoubleRow)
                    if ic == 0:
                        with nc.allow_low_precision(reason="fp16 k"):
                            nc.vector.tensor_scalar(
                                out=k_res[:, mt, ts0:ts0 + 512], in0=ps[:],
                                scalar1=INV_S, scalar2=bk_sb[:, mt:mt + 1],
                                op0=mybir.AluOpType.mult,
                                op1=mybir.AluOpType.add)
                    else:
                        nc.scalar.activation(
                            k_res[:, mt, ts0:ts0 + 512], ps[:],
                            mybir.ActivationFunctionType.Identity,
                            bias=bk_sb[:, mt:mt + 1], scale=INV_S)
                    rope_inplace(k_res[:, mt, ts0:ts0 + 512],
                                 k_res[:, mt, ts0:ts0 + 512], ts0,
                                 dq=nc.scalar if ic == 0 else None)

            def proj_v():
                v_terms = ((xb8, wv_t[0]), (xb8, wv_t[1]), (xb8r, wv_t[0]))
                if ic == 0:
                    # pair-major on psS (idle before attention); term-major
                    # with the w-residual term last, since the residual
                    # weight chunks are the last DMAs to land
                    ps_v = [psA.tile([P, M], F32, tag="psA", name=f"psv{t}")
                            for t in range(2)]
                    ps_v += [psV.tile([P, M], F32, tag="psV", name=f"psv{t}")
                             for t in range(2, 4)]
                    vt0 = ((xb8, wv_t[0]), (xb8r, wv_t[0]), (xb8, wv_t[1]))
                    for ti, (xt, wt) in enumerate(vt0):
                        for a in range(NP):
                            for tt in range(4):
                                nc.tensor.matmul(
                                    ps_v[tt][:],
                                    lhsT=xt[:, 2 * a:2 * a + 2,
                                            tt * P:(tt + 1) * P],
                                    rhs=wt[:, 2 * a:2 * a + 2, :],
                                    start=(a == 0 and ti == 0),
                                    stop=(a == NP - 1 and ti == 2),
                                    perf_mode=mybir.MatmulPerfMode.DoubleRow)
                    for tt in range(4):
                        nc.vector.scalar_tensor_tensor(
                            out=v_res[:, 4 * ic + tt, :], in0=ps_v[tt][:],
                            scalar=INV_S, in1=bvb_sb[:],
                            op0=mybir.AluOpType.mult, op1=mybir.AluOpType.add)
                    return
                for tt in range(4):
                    ps = psA.tile([P, M], F32, tag="psA")
                    for ti, (xt, wt) in enumerate(v_terms):
                        for a in range(NP):
                            nc.tensor.matmul(
                                ps[:],
                                lhsT=xt[:, 2 * a:2 * a + 2,
                                        tt * P:(tt + 1) * P],
                                rhs=wt[:, 2 * a:2 * a + 2, :],
                                start=(a == 0 and ti == 0),
                                stop=(a == NP - 1 and ti == 2),
                                perf_mode=mybir.MatmulPerfMode.DoubleRow)
                    nc.vector.scalar_tensor_tensor(
                        out=v_res[:, 4 * ic + tt, :], in0=ps[:],
                        scalar=INV_S, in1=bvb_sb[:],
                        op0=mybir.AluOpType.mult, op1=mybir.AluOpType.add)

            if ic == 0:
                # wk lands last on the SP queue: fill the gap with proj v
                proj_v()
                _mark(f"v{ic}")
                proj_k()
                _mark(f"k{ic}")
            else:
                proj_k()
                _mark(f"k{ic}")
                if ic == NT - 1:
                    # att3 is Act-exp-capacity-bound while the projections
                    # are Act-idle: pull head-0's first off-diag score pairs
                    # forward so their exps overlap the v projection
                    for pj in range(4):
                        jt0, jt1 = 2 * pj, 2 * pj + 1
                        ps2p = psS.tile([P, 2, 512], F32, tag="psS2")
                        nc.tensor.matmul(
                            ps2p[:, 0, :],
                            lhsT=k_res[:, 0, jt0 * P:(jt0 + 1) * P],
                            rhs=qcur[:, 0, :], start=True, stop=True)
                        nc.tensor.matmul(
                            ps2p[:, 1, :],
                            lhsT=k_res[:, 0, jt1 * P:(jt1 + 1) * P],
                            rhs=qcur[:, 0, :], start=True, stop=True)
                        ex2p = expool.tile([P, 2, 512], F16, tag="ex")
                        nc.scalar.activation(
                            ex2p[:, :, :], ps2p[:, :, :],
                            mybir.ActivationFunctionType.Exp,
                            bias=eb_sb[:, 0:1], scale=SCALE)
                        pre_ex[pj] = ex2p
                proj_v()
                _mark(f"v{ic}")

            if ic + 1 < NT:
                x_next = load_x(ic + 1)

            if ic == 0:
                # wo is first needed by ph3(0) woven into attention(1);
                # dispatch after x1 so it doesn't steal DMA bandwidth from
                # the wv/wk/x1 loads the pipeline stalls on. Sync queue: the
                # scalar queue is head-of-line blocked on chunk-0 rope
                # shuffles; the gpsimd SWDGE path trickles.
                for j in range(H_LOC):
                    nc.sync.dma_start(out=wo_b[:, j:j + 1, :],
                                      in_=wor[:, j:j + 1, :])
                    nc.sync.dma_start(out=wor_b[:, j:j + 1, :],
                                      in_=worr[:, j:j + 1, :])

            # ---- attention for chunk ic (weaving ph3 of chunk ic-1) ----
            attn8 = atpool.tile([P, H_LOC, 512], F8, tag="attn8")
            attn8r = atpool.tile([P, H_LOC, 512], F8, tag="attn8r")
            njt = 4 * ic + 4
            slots_left = H_LOC * njt

            def emit_chain(dfr):
                """Deferred per-head softmax-normalize chain: recip + fp8
                attn split. Deferred into the NEXT head's tile loop so the
                in-order DVE queue never parks waiting for the Pool
                all_reduce (which sits behind woven ph3 evicts)."""
                rden_d, ps_av_d, hh = dfr
                with nc.allow_low_precision(reason="softmax reciprocal"):
                    if rden_d.space == tile.bass.MemorySpace.PSUM:
                        den_ps = rden_d
                        rden_d = rdpool.tile([P, 512], F16, tag="rden")
                        nc.vector.reciprocal(rden_d[:], den_ps[:, 0, :])
                    else:
                        nc.vector.reciprocal(rden_d[:], rden_d[:])
                    nc.vector.scalar_tensor_tensor(
                        out=attn8[:, hh, :], in0=ps_av_d[:], scalar=SA,
                        in1=rden_d[:],
                        op0=mybir.AluOpType.mult, op1=mybir.AluOpType.mult)
                    a16 = a16pool.tile([P, 512], F16, tag="a16")
                    nc.vector.scalar_tensor_tensor(
                        out=a16[:], in0=ps_av_d[:], scalar=SA, in1=rden_d[:],
                        op0=mybir.AluOpType.mult, op1=mybir.AluOpType.mult)
                    eng = nc.vector if hh == H_LOC - 1 else nc.gpsimd
                    eng.tensor_tensor(
                        attn8r[:, hh, :], a16[:], attn8[:, hh, :],
                        mybir.AluOpType.subtract)

            deferred = None
            ic0_chains = []
            n_items = CT * 6
            slots_done = 0
            for h in range(H_LOC):
                ps_av = psV.tile([P, 512], F32, tag="psV")
                exsum = espool.tile([P, 512], F16, tag="exsum")
                prev = None  # (ex2 tile, c00, c01, jt0) awaiting av matmuls
                for pj in range(njt // 2):
                    jt0, jt1 = 2 * pj, 2 * pj + 1
                    d0, d1 = jt0 - 4 * ic, jt1 - 4 * ic
                    c00 = 128 * d0 if d0 > 0 else 0
                    c01 = 128 * d1 if d1 > 0 else 0
                    if pj == min(2, njt // 2 - 1) and deferred is not None:
                        emit_chain(deferred)
                        deferred = None
                    if h == 0 and pj in pre_ex:
                        # precomputed during the projection phase
                        ex2 = pre_ex[pj]
                        slots_done += 2
                        if pending is not None:
                            eff = slots_left + (36 if ic == NT - 1 else 0)
                            tgt = min(n_items,
                                      (n_items * slots_done) // eff)
                            pending.step(tgt - pending.pos)
                        if prev is not None:
                            pex2, pc00, pc01, pjt0 = prev
                            nc.tensor.matmul(
                                ps_av[:, pc00:],
                                lhsT=v_res[:, pjt0, h * HS:(h + 1) * HS],
                                rhs=pex2[:, 0, pc00:],
                                start=(pjt0 == 0), stop=False,
                                skip_group_check=True)
                            nc.tensor.matmul(
                                ps_av[:, pc01:],
                                lhsT=v_res[:, pjt0 + 1, h * HS:(h + 1) * HS],
                                rhs=pex2[:, 1, pc01:],
                                start=False, stop=False,
                                skip_group_check=True)
                        with nc.allow_low_precision(reason="fp16 denom"):
                            if pj == 0:
                                nc.vector.tensor_tensor(
                                    exsum[:], ex2[:, 0, :], ex2[:, 1, :],
                                    mybir.AluOpType.add)
                            else:
                                nc.vector.tensor_tensor(
                                    exsum[:], exsum[:],
                                    ex2[:, 0, :], mybir.AluOpType.add)
                                nc.vector.tensor_tensor(
                                    exsum[:], exsum[:],
                                    ex2[:, 1, :], mybir.AluOpType.add)
                        prev = (ex2, 0, 0, jt0)
                        continue
                    # scores for a PAIR of key tiles into one 2-bank PSUM
                    # tile; one Act exp over both planes amortizes the
                    # fixed per-op Act overhead (the attention pacer)
                    ps2 = psS.tile([P, 2, 512], F32, tag="psS2")
                    nc.tensor.matmul(
                        ps2[:, 0, c00:],
                        lhsT=k_res[:, h, jt0 * P:(jt0 + 1) * P],
                        rhs=qcur[:, h, c00:],
                        start=True, stop=True)
                    split = d0 >= 0 and ic < NT - 1
                    # diagonal pairs in PE-bound windows: exact-trim plane 1
                    # and pay a second (smaller) exp on the slack Act engine.
                    # Otherwise plane 1 also starts at c00: cols [c00:c01]
                    # are real (causal-masked) scores so the shared exp
                    # never reads uninitialized PSUM
                    c1lo = c01 if split else c00
                    nc.tensor.matmul(
                        ps2[:, 1, c1lo:],
                        lhsT=k_res[:, h, jt1 * P:(jt1 + 1) * P],
                        rhs=qcur[:, h, c1lo:],
                        start=True, stop=True)
                    ex2 = expool.tile([P, 2, 512], F16, tag="ex")
                    if split:
                        nc.scalar.activation(
                            ex2[:, 0, c00:], ps2[:, 0, c00:],
                            mybir.ActivationFunctionType.Exp,
                            bias=eb_sb[:, 0:1], scale=SCALE)
                        nc.scalar.activation(
                            ex2[:, 1, c01:], ps2[:, 1, c01:],
                            mybir.ActivationFunctionType.Exp,
                            bias=eb_sb[:, 0:1], scale=SCALE)
                    else:
                        nc.scalar.activation(
                            ex2[:, :, c00:], ps2[:, :, c00:],
                            mybir.ActivationFunctionType.Exp,
                            bias=eb_sb[:, 0:1], scale=SCALE)
                    # causal triangle: for every diagonal tile the global
                    # query base ts0+c0 equals the key base jt*P, so one
                    # [P,P] keep-where-col>=row mask serves them all
                    if d0 >= 0:
                        nc.vector.tensor_tensor(
                            ex2[:, 0, c00:c00 + P], ex2[:, 0, c00:c00 + P],
                            tri_sb[:], mybir.AluOpType.mult)
                    if d1 >= 0:
                        nc.vector.tensor_tensor(
                            ex2[:, 1, c01:c01 + P], ex2[:, 1, c01:c01 + P],
                            tri_sb[:], mybir.AluOpType.mult)
                    slots_done += 2
                    if pending is not None:
                        # spread ph3 items evenly across the window; on the
                        # last chunk hold ~16 back so they bridge the final
                        # head's normalize-chain latency after the window
                        eff = slots_left + (36 if ic == NT - 1 else 0)
                        tgt = min(n_items, (n_items * slots_done) // eff)
                        pending.step(tgt - pending.pos)
                    elif qw is not None:
                        nqw = len(qw.items)
                        qw.step((nqw * slots_done) // slots_left - qw.pos)
                    if prev is not None:
                        pex2, pc00, pc01, pjt0 = prev
                        nc.tensor.matmul(
                            ps_av[:, pc00:],
                            lhsT=v_res[:, pjt0, h * HS:(h + 1) * HS],
                            rhs=pex2[:, 0, pc00:],
                            start=(pjt0 == 0), stop=False,
                            skip_group_check=True)
                        nc.tensor.matmul(
                            ps_av[:, pc01:],
                            lhsT=v_res[:, pjt0 + 1, h * HS:(h + 1) * HS],
                            rhs=pex2[:, 1, pc01:],
                            start=False, stop=False,
                            skip_group_check=True)
                    with nc.allow_low_precision(reason="fp16 softmax denom"):
                        if pj == 0:
                            if ic == 0:
                                nc.vector.tensor_copy(out=exsum[:],
                                                      in_=ex2[:, 0, :])
                                nc.vector.tensor_tensor(
                                    exsum[:, c01:], exsum[:, c01:],
                                    ex2[:, 1, c01:], mybir.AluOpType.add)
                            else:
                                nc.vector.tensor_tensor(
                                    exsum[:], ex2[:, 0, :], ex2[:, 1, :],
                                    mybir.AluOpType.add)
                        else:
                            nc.vector.tensor_tensor(
                                exsum[:, c00:], exsum[:, c00:],
                                ex2[:, 0, c00:], mybir.AluOpType.add)
                            nc.vector.tensor_tensor(
                                exsum[:, c01:], exsum[:, c01:],
                                ex2[:, 1, c01:], mybir.AluOpType.add)
                    prev = (ex2, c00, c01, jt0)
                pex2, pc00, pc01, pjt0 = prev
                nc.tensor.matmul(
                    ps_av[:, pc00:],
                    lhsT=v_res[:, pjt0, h * HS:(h + 1) * HS],
                    rhs=pex2[:, 0, pc00:],
                    start=(pjt0 == 0), stop=False,
                    skip_group_check=True)
                nc.tensor.matmul(
                    ps_av[:, pc01:],
                    lhsT=v_res[:, pjt0 + 1, h * HS:(h + 1) * HS],
                    rhs=pex2[:, 1, pc01:],
                    start=False, stop=True,
                    skip_group_check=True)
                if (h == H_LOC - 1 and ic == NT - 1) or ic == 0:
                    # chunk 0 (PE idle, DVE-bound) and end of kernel (PE
                    # free, chain gates the output projection): sum the
                    # denominator with a ones-matmul instead of the slower
                    # Pool all_reduce
                    ps_d = psS.tile([P, 2, 512], F32, tag="psS2")
                    nc.tensor.matmul(
                        ps_d[:, 0, :], lhsT=ones_sb[:], rhs=exsum[:],
                        start=True, stop=True)
                    deferred = (ps_d, ps_av, h)
                else:
                    rden = rdpool.tile([P, 512], F16, tag="rden")
                    nc.gpsimd.partition_all_reduce(
                        rden[:], exsum[:], channels=P,
                        reduce_op=bass_isa.ReduceOp.add)
                    deferred = (rden, ps_av, h)
                if h == H_LOC - 1 and ic < NT - 1:
                    emit_chain(deferred)
                    deferred = None

            _mark(f"att{ic}")
            for dfr in ic0_chains:
                emit_chain(dfr)
            if pending is not None:
                pending.finish()
            _mark(f"ph3fin{ic}")
            pending = Ph3(ic, attn8, attn8r,
                          pools=[psA, psV] if ic == NT - 1 else None)
            if deferred is not None:
                # last chunk: co0's three g0 steps only read heads 0-1; emit
                # them BEFORE the last head's normalize chain so they don't
                # inherit a wait on its attn8 write (sem counts are
                # emission-order conservative) and bridge its latency
                pending.step(6)
                emit_chain(deferred)
                deferred = None

        pending.finish()
        _mark("ph3last")

    nc.finalize()
    return nc


def get_nc(phases=(1, 2, 3)):
    if phases not in _NC_CACHE:
        _NC_CACHE[phases] = _build(phases)
    return _NC_CACHE[phases]


def _rope_tables():
    inv_freq = 1.0 / (BASE ** (np.arange(0, ROT, 2, dtype=np.float64) / ROT))
    freqs = np.arange(T, dtype=np.float64)[:, None] * inv_freq[None, :]  # [T, 32]
    cos_h = np.cos(freqs).T.astype(np.float32)   # [32, T]
    sin_h = np.sin(freqs).T.astype(np.float32)
    cosT = np.concatenate([cos_h, cos_h], axis=0)          # [64, T]
    nsT = np.concatenate([-sin_h, sin_h], axis=0)          # [64, T] signed sin
    return (np.ascontiguousarray(cosT.astype(np.float16)),
            np.ascontiguousarray(nsT.astype(np.float16)))


def _split8(a, s):
    """a*s = a8 + a8r (both fp8 e4m3) up to second-order quantization."""
    scaled = a * np.float32(s)
    a8 = scaled.astype(NPF8)
    a8r = (scaled - a8.astype(np.float32)).astype(NPF8)
    return np.ascontiguousarray(a8), np.ascontiguousarray(a8r)


def make_in_maps(x, Wq, bq, Wk, bk, Wv, bv, Wo, bo):
    cosT, nsT = _rope_tables()
    in_maps = []
    for c in range(N_CORES):
        b, g = divmod(c, TPG)
        ms = slice(g * M, (g + 1) * M)
        xh8, xh8r = _split8(x[b].T, SX)
        wq8, wq8r = _split8(Wq[ms].T, SW)
        wk8, wk8r = _split8(Wk[ms].T, SW)
        wv8, wv8r = _split8(Wv[ms].T, SW)
        wo8, wo8r = _split8(Wo[:, ms].T, SWO)
        in_maps.append({
            "xh8": xh8, "xh8r": xh8r,
            "wq8": wq8, "wq8r": wq8r,
            "wk8": wk8, "wk8r": wk8r,
            "wv8": wv8, "wv8r": wv8r,
            "wo8T": wo8, "wo8rT": wo8r,
            "bqc": np.ascontiguousarray(bq[ms].reshape(H_LOC, P).T),
            "bkc": np.ascontiguousarray(bk[ms].reshape(H_LOC, P).T),
            "bvr": np.ascontiguousarray(bv[ms].reshape(1, M).astype(np.float16)),
            "ebias": np.full((P, 1), EXP_BIAS, np.float32),
            "trimask": np.triu(np.ones((P, P), np.float16)),
            "cosT": cosT,
            "nsT": nsT,
        })
    return in_maps


def assemble(results, bo):
    out = np.empty((B, T, C), dtype=np.float32)
    for b in range(B):
        acc = results[b * TPG]["outT"].astype(np.float32)
        for g in range(1, TPG):
            acc = acc + results[b * TPG + g]["outT"].astype(np.float32)
        out[b] = acc.T + bo[None, :]
    return out


def kernel(x, Wq, bq, Wk, bk, Wv, bv, Wo, bo):
    nc = get_nc()
    in_maps = make_in_maps(np.asarray(x, np.float32),
                           np.asarray(Wq, np.float32), np.asarray(bq, np.float32),
                           np.asarray(Wk, np.float32), np.asarray(bk, np.float32),
                           np.asarray(Wv, np.float32), np.asarray(bv, np.float32),
                           np.asarray(Wo, np.float32), np.asarray(bo, np.float32))
    res = run_bass_kernel_spmd(nc, in_maps, list(range(N_CORES)))
    return assemble(res.results, np.asarray(bo, np.float32))



# revision 95
# speedup vs baseline: 1.0072x; 1.0064x over previous
"""Trainium2 Bass kernel for a full causal MHA layer (B=2, T=2048, C=2048, H=16,
partial RoPE on first 64 dims of each 128-dim head).

Sharding over 8 cores: core c handles batch b=c//4 and heads [4g, 4g+4), g=c%4
(tensor-parallel over heads x data-parallel over batch).

Fully fused single pass per core, fp16 data plane (fp32 PSUM accumulation):
  for each 512-token chunk ic:
    proj q/k (fp16 weights stationary, fp16 x moving), bias+partial-RoPE,
      q and k stay resident in SBUF (no DRAM spills)
    proj v -> v_res [key, jt, m] fp16 resident
    attention for chunk ic over heads h: per key-tile jt
      scoresT[k,q] (k_res stationary fp16, q moving fp16)
      -> exp(scale*s - 10*ln2) -> ex fp16 (Act), triangle mask on diagonal
         tiles only (DVE mult by a const [128,128] triu mask), exact causal
         col-trimming
      -> av accumulation outT[d,q] via PE; softmax denominator via DVE
         exsum adds + gpsimd partition_all_reduce (no PE ones-matmuls)
      output-projection matmuls of chunk ic-1 are woven between attention
      matmuls to keep PE busy during Act-latency windows
    phase3(ic): out partial outT[c,q] = sum_mt woT attn, DVE evict fp16, DMA
Host: slices inputs per core (fp16), sums the 4 TP partials per batch + bo.
"""

import math

import ml_dtypes
import numpy as np

NPF8 = ml_dtypes.float8_e4m3

import concourse.bass_isa as bass_isa
import concourse.mybir as mybir
import concourse.tile as tile
from concourse import bacc
from concourse.bass_utils import run_bass_kernel_spmd

F32 = mybir.dt.float32
F16 = mybir.dt.float16
F8 = mybir.dt.float8e4

B, T, C = 2, 2048, 2048
H = 16
HS = 128
ROT = 64
HALF = 32
BASE = 10000.0

N_CORES = 8
TPG = 4                # TP group size (heads split)
H_LOC = H // TPG       # 4 heads per core
M = H_LOC * HS         # 512 local head-dim columns
SCALE = 1.0 / math.sqrt(HS)
EXP_BIAS = -10.0 * math.log(2.0)   # exp(s*SCALE - 10ln2): keeps fp16 in range
SX = 16.0                 # fp8 quantization scale for x
SW = 1024.0               # fp8 quantization scale for Wq/Wk/Wv
INV_S = 1.0 / (SX * SW)   # folded into the projection evictions
SWO = 1024.0              # fp8 quantization scale for Wo
SA = 64.0                 # fp8 quantization scale for attn outputs
INV_SO = 1.0 / (SWO * SA)  # folded into the output-proj evictions

P = 128
NT = T // 512          # 4 t-chunks of 512
CT = C // P            # 16 contraction tiles
JT = T // P            # 16 key tiles per head

_NC_CACHE = {}
PHASE_MARKS = []  # (label, last-emitted instruction name); debug aid only


def _build(phases=(1, 2, 3)):
    nc = bacc.Bacc(None, target_bir_lowering=False)
    PHASE_MARKS.clear()

    def _mark(label):
        # consumes one instruction name as a monotonic position marker
        PHASE_MARKS.append((label, nc.get_next_instruction_name()))

    xh8 = nc.declare_dram_parameter("xh8", [C, T], F8, isOutput=False)
    xh8r = nc.declare_dram_parameter("xh8r", [C, T], F8, isOutput=False)
    wq8 = nc.declare_dram_parameter("wq8", [C, M], F8, isOutput=False)
    wq8r = nc.declare_dram_parameter("wq8r", [C, M], F8, isOutput=False)
    wk8 = nc.declare_dram_parameter("wk8", [C, M], F8, isOutput=False)
    wk8r = nc.declare_dram_parameter("wk8r", [C, M], F8, isOutput=False)
    wv8 = nc.declare_dram_parameter("wv8", [C, M], F8, isOutput=False)
    wv8r = nc.declare_dram_parameter("wv8r", [C, M], F8, isOutput=False)
    wo8T = nc.declare_dram_parameter("wo8T", [M, C], F8, isOutput=False)
    wo8rT = nc.declare_dram_parameter("wo8rT", [M, C], F8, isOutput=False)
    bqc = nc.declare_dram_parameter("bqc", [P, H_LOC], F32, isOutput=False)
    bkc = nc.declare_dram_parameter("bkc", [P, H_LOC], F32, isOutput=False)
    bvr = nc.declare_dram_parameter("bvr", [1, M], F16, isOutput=False)
    ebias = nc.declare_dram_parameter("ebias", [P, 1], F32, isOutput=False)
    trimask = nc.declare_dram_parameter("trimask", [P, P], F16, isOutput=False)
    cosT = nc.declare_dram_parameter("cosT", [ROT, T], F16, isOutput=False)
    nsT = nc.declare_dram_parameter("nsT", [ROT, T], F16, isOutput=False)
    outT = nc.declare_dram_parameter("outT", [C, T], F16, isOutput=True)

    with tile.TileContext(nc) as tc, \
         tc.tile_pool(name="const", bufs=1) as const, \
         tc.tile_pool(name="xp", bufs=CT) as xpool, \
         tc.tile_pool(name="qc", bufs=2) as qpool, \
         tc.tile_pool(name="at", bufs=2) as atpool, \
         tc.tile_pool(name="rp", bufs=6) as rpool, \
         tc.tile_pool(name="exp", bufs=5) as expool, \
         tc.tile_pool(name="exs", bufs=3) as espool, \
         tc.tile_pool(name="rd", bufs=5) as rdpool, \
         tc.tile_pool(name="a16", bufs=2) as a16pool, \
         tc.tile_pool(name="oe", bufs=3) as oepool, \
         tc.tile_pool(name="psA", bufs=2, space="PSUM") as psA, \
         tc.tile_pool(name="psS", bufs=2, space="PSUM") as psS, \
         tc.tile_pool(name="psV", bufs=2, space="PSUM") as psV:

        cos_sb = const.tile([ROT, T], F16, tag="cos")
        ns_sb = const.tile([ROT, T], F16, tag="ns")
        bq_sb = const.tile([P, H_LOC], F32, tag="bq")
        bk_sb = const.tile([P, H_LOC], F32, tag="bk")
        bvb_sb = const.tile([P, M], F16, tag="bvb")
        eb_sb = const.tile([P, 1], F32, tag="ebias")
        tri_sb = const.tile([P, P], F16, tag="trimask")
        k_res = const.tile([P, H_LOC, T], F16, tag="kres")
        v_res = const.tile([P, JT, M], F16, tag="vres")
        wq_t = [const.tile([P, CT, M], F8, tag=f"wq{i}", name=f"wq{i}")
                for i in range(2)]
        wk_t = [const.tile([P, CT, M], F8, tag=f"wk{i}", name=f"wk{i}")
                for i in range(2)]
        wv_t = [const.tile([P, CT, M], F8, tag=f"wv{i}", name=f"wv{i}")
                for i in range(2)]
        wo_b = const.tile([P, H_LOC, C], F8, tag="wob")
        wor_b = const.tile([P, H_LOC, C], F8, tag="worb")
        wre = [d[:].rearrange("(ct p) m -> p ct m", p=P)
               for d in (wq8, wq8r, wk8, wk8r, wv8, wv8r)]
        wqre, wqrre, wkre, wkrre, wvre, wvrre = wre
        wor = wo8T[:].rearrange("(mt p) c -> p mt c", p=P)
        worr = wo8rT[:].rearrange("(mt p) c -> p mt c", p=P)

        xr = xh8[:].rearrange("(ct p) t -> p ct t", p=P)
        xrr = xh8r[:].rearrange("(ct p) t -> p ct t", p=P)
        otr = outT[:].rearrange("(co p) t -> p co t", p=P)

        def load_x(ic):
            ts0 = ic * 512
            xb = xpool.tile([P, CT, 512], F8, tag="xb8", name=f"xb{ic}", bufs=2)
            xbr = xpool.tile([P, CT, 512], F8, tag="xb8r", name=f"xbr{ic}",
                             bufs=2)
            for j in range(4):
                nc.sync.dma_start(out=xb[:, 4 * j:4 * j + 4, :],
                                  in_=xr[:, 4 * j:4 * j + 4, ts0:ts0 + 512])
                nc.sync.dma_start(out=xbr[:, 4 * j:4 * j + 4, :],
                                  in_=xrr[:, 4 * j:4 * j + 4, ts0:ts0 + 512])
            return xb, xbr

        # startup, ordered by first use: main-term operands (x8 + wq8) first,
        # then the residual streams, then wv (main, res), wk (main, res).
        # x on the SP queue, wq on the Act queue so the two interleave on the
        # DMA engines; chunked so HWDGE overheads don't pace it.
        xb0 = xpool.tile([P, CT, 512], F8, tag="xb8", name="xb0", bufs=2)
        xb0r = xpool.tile([P, CT, 512], F8, tag="xb8r", name="xb0r", bufs=2)
        # first ct-pair as its own small DMA so the very first matmul can
        # start as early as possible
        nc.sync.dma_start(out=xb0[:, 0:2, :], in_=xr[:, 0:2, 0:512])
        nc.scalar.dma_start(out=wq_t[0][:, 0:2, :], in_=wqre[:, 0:2, :])
        nc.sync.dma_start(out=xb0[:, 2:4, :], in_=xr[:, 2:4, 0:512])
        nc.scalar.dma_start(out=wq_t[0][:, 2:4, :], in_=wqre[:, 2:4, :])
        for j in range(1, 4):
            a, b = 4 * j, 4 * j + 4
            nc.sync.dma_start(out=xb0[:, a:b, :], in_=xr[:, a:b, 0:512])
            nc.scalar.dma_start(out=wq_t[0][:, a:b, :], in_=wqre[:, a:b, :])
        for j in range(4):
            a, b = 4 * j, 4 * j + 4
            nc.sync.dma_start(out=xb0r[:, a:b, :], in_=xrr[:, a:b, 0:512])
            nc.scalar.dma_start(out=wq_t[1][:, a:b, :], in_=wqrre[:, a:b, :])
        for j in range(4):
            a, b = 4 * j, 4 * j + 4
            nc.sync.dma_start(out=wv_t[0][:, a:b, :], in_=wvre[:, a:b, :])
        for j in range(4):
            a, b = 4 * j, 4 * j + 4
            nc.sync.dma_start(out=wv_t[1][:, a:b, :], in_=wvrre[:, a:b, :])
        for j in range(4):
            a, b = 4 * j, 4 * j + 4
            nc.sync.dma_start(out=wk_t[0][:, a:b, :], in_=wkre[:, a:b, :])
        for j in range(4):
            a, b = 4 * j, 4 * j + 4
            nc.sync.dma_start(out=wk_t[1][:, a:b, :], in_=wkrre[:, a:b, :])
        ones_sb = const.tile([P, P], F16, tag="ones")
        nc.gpsimd.memset(ones_sb[:], 1.0)
        nc.gpsimd.dma_start(out=bq_sb[:], in_=bqc[:])
        nc.gpsimd.dma_start(out=bk_sb[:], in_=bkc[:])
        nc.gpsimd.dma_start(out=cos_sb[:], in_=cosT[:])
        nc.gpsimd.dma_start(out=ns_sb[:], in_=nsT[:])
        nc.gpsimd.dma_start(out=eb_sb[:], in_=ebias[:])
        nc.gpsimd.dma_start(out=tri_sb[:], in_=trimask[:])
        nc.gpsimd.dma_start(out=bvb_sb[:], in_=bvr[0:1, :].to_broadcast([P, M]))

        NP = CT // 2   # 8 ct-pairs per contraction

        def fb_terms(w_t, xb, xbr):
            """(lhsT_tile, rhs_tile) per error-feedback term: main, w-res,
            x-res. All DoubleRow fp8 over ct-pairs."""
            return ((w_t[0], xb), (w_t[1], xb), (w_t[0], xbr))

        def rope_inplace(dst, tmp_src, ts0, dq=None):
            """dst[0:ROT, 512] fp16 <- rope(tmp_src rows 0:ROT) in place.
            tmp_src rows are pre-rope biased values; dst may alias tmp_src.
            dq picks the DMA queue for the partition-swap (the SP queue is
            backed up with weight loads during chunk 0)."""
            dq = dq or nc.sync
            sh = rpool.tile([ROT, 512], F16, tag="sh")
            dq.dma_start(out=sh[0:HALF], in_=tmp_src[HALF:ROT])
            dq.dma_start(out=sh[HALF:ROT], in_=tmp_src[0:HALF])
            rot = rpool.tile([ROT, 512], F16, tag="rot")
            nc.vector.tensor_tensor(rot[:], sh[:], ns_sb[:, ts0:ts0 + 512],
                                    mybir.AluOpType.mult)
            tcos = rpool.tile([ROT, 512], F16, tag="tcos")
            nc.vector.tensor_tensor(tcos[:], tmp_src[:ROT], cos_sb[:, ts0:ts0 + 512],
                                    mybir.AluOpType.mult)
            nc.vector.tensor_tensor(dst[0:ROT], tcos[:], rot[:],
                                    mybir.AluOpType.add)

        class Ph3:
            """Output projection for chunk ic (fp8 DoubleRow, 3-term error
            feedback); matmuls are dispensed one at a time (step) so they
            weave between attention matmuls. 6 steps per co:
            (term, g) with term in {wo8*a8, wo8r*a8, wo8*a8r}, g the DR pair
            of mt planes."""

            def __init__(self, ic, a8, a8r, pools=None):
                self.ic = ic
                self.terms = ((wo_b, a8), (wor_b, a8), (wo_b, a8r))
                self.items = [(co, s) for co in range(CT) for s in range(6)]
                if ic == NT - 1:
                    # stagger co0/co1: their g0 steps (heads 0-1 only) first,
                    # g1 steps after — bridges the last head's normalize
                    # chain at the attention/finish boundary
                    self.items = (
                        [(0, 0), (0, 1), (0, 2), (1, 0), (1, 1), (1, 2),
                         (0, 3), (0, 4), (0, 5), (1, 3), (1, 4), (1, 5)]
                        + [(co, s) for co in range(2, CT) for s in range(6)])
                self.pos = 0
                self.ps = None
                self.pools = pools or [psA]
                self.finishing = False

            def step(self, n=1):
                for _ in range(n):
                    if self.pos >= len(self.items):
                        return
                    co, s = self.items[self.pos]
                    self.pos += 1
                    # g-major: the three g=0 steps only read heads 0-1,
                    # which are normalized well before heads 2-3 land
                    g, t = divmod(s, 3)
                    if s == 0:
                        pool = self.pools[co % len(self.pools)]
                        ps = pool.tile([P, 512], F32,
                                       tag="psA" if pool is psA else "psV",
                                       name=f"ps3_{self.ic}_{co}")
                        self.ps_by_co = getattr(self, "ps_by_co", {})
                        self.ps_by_co[co] = ps
                    self.ps = self.ps_by_co[co]
                    wt, at = self.terms[t]
                    nc.tensor.matmul(
                        self.ps[:],
                        lhsT=wt[:, 2 * g:2 * g + 2, co * P:(co + 1) * P],
                        rhs=at[:, 2 * g:2 * g + 2, :],
                        start=(s == 0), stop=(s == 5),
                        perf_mode=mybir.MatmulPerfMode.DoubleRow)
                    if s == 5:
                        if co % 4 == 0:
                            self.ot = oepool.tile([P, 4, 512], F16, tag="ot")
                        last = self.ic == NT - 1
                        if self.finishing and last and co % 2 == 0:
                            # end of kernel: DVE is idle, alternate with Act
                            # so the final evict drain is 2-wide
                            with nc.allow_low_precision(reason="fp16 out"):
                                nc.vector.tensor_scalar(
                                    out=self.ot[:, co % 4, :], in0=self.ps[:],
                                    scalar1=INV_SO, scalar2=None,
                                    op0=mybir.AluOpType.mult)
                        elif self.finishing:
                            # post-attention block: DVE is draining attention
                            # tail work, evict on Act
                            nc.scalar.activation(
                                self.ot[:, co % 4, :], self.ps[:],
                                mybir.ActivationFunctionType.Identity,
                                scale=INV_SO)
                        elif co % 3 == 2:
                            nc.scalar.activation(
                                self.ot[:, co % 4, :], self.ps[:],
                                mybir.ActivationFunctionType.Identity,
                                scale=INV_SO)
                        else:
                            # woven between attention matmuls: mostly DVE
                            # with Act picking up every third (gpsimd can't
                            # read PSUM)
                            with nc.allow_low_precision(reason="fp16 out"):
                                nc.vector.tensor_scalar(
                                    out=self.ot[:, co % 4, :], in0=self.ps[:],
                                    scalar1=INV_SO, scalar2=None,
                                    op0=mybir.AluOpType.mult)
                        # last chunk: per-co DMAs on alternating queues so
                        # the post-matmul tail is one small empty-pipe
                        # transfer, not a backlog of big ones
                        step = (1 if co >= 12 else 2) if last else 2
                        if co % step == step - 1:
                            j0 = co % 4 - (step - 1)
                            # co14/15 go on sync: its DMA dispatch is half
                            # the scalar queue's, and it's empty by then
                            dq = (nc.sync if co >= 14 else nc.scalar) \
                                if (last and co % 2 == 1) else nc.sync
                            dq.dma_start(
                                out=otr[:, co - step + 1:co + 1,
                                        self.ic * 512:self.ic * 512 + 512],
                                in_=self.ot[:, j0:j0 + step, :])

            def finish(self):
                self.finishing = True
                self.step(len(self.items) - self.pos)

        class Qweave:
            """q-projection of chunk 1, head-tiles mt0-mt1, woven a few
            matmuls at a time into attention(0)'s gaps (no ph3 exists yet,
            and attention(0) is DVE-chain-bound)."""

            N_MT = 2

            def __init__(self, xb1, xb1r):
                self.qcur = qpool.tile([P, H_LOC, 512], F16, tag="qcur",
                                       name="qcur1")
                self.terms = ((wq_t[0], xb1), (wq_t[1], xb1), (wq_t[0], xb1r))
                self.items = [(mt, ti, a) for mt in range(self.N_MT)
                              for ti in range(3) for a in range(NP)]
                self.pos = 0
                self.ps = None

            def step(self, n=1):
                for _ in range(n):
                    if self.pos >= len(self.items):
                        return
                    mt, ti, a = self.items[self.pos]
                    self.pos += 1
                    if ti == 0 and a == 0:
                        self.ps = psA.tile([P, 512], F32, tag="psA",
                                           name=f"psqw{mt}")
                    wt, xt = self.terms[ti]
                    nc.tensor.matmul(
                        self.ps[:],
                        lhsT=wt[:, 2 * a:2 * a + 2, mt * P:(mt + 1) * P],
                        rhs=xt[:, 2 * a:2 * a + 2, :],
                        start=(ti == 0 and a == 0),
                        stop=(ti == 2 and a == NP - 1),
                        perf_mode=mybir.MatmulPerfMode.DoubleRow)
                    if ti == 2 and a == NP - 1:
                        nc.scalar.activation(
                            self.qcur[:, mt, :], self.ps[:],
                            mybir.ActivationFunctionType.Identity,
                            bias=bq_sb[:, mt:mt + 1], scale=INV_S)
                        rope_inplace(self.qcur[:, mt, :],
                                     self.qcur[:, mt, :], 512)

        pending = None
        qw = None

        for ic in range(NT):
            ts0 = ic * 512
            x_cur = (xb0, xb0r) if ic == 0 else x_next

            # ---- proj q ----
            _mark(f"pre_q{ic}")
            pre_ex = {}
            if ic == 1 and qw is not None:
                qcur = qw.qcur    # first tiles computed during attention(0)
                mt_start = qw.N_MT
            else:
                qcur = qpool.tile([P, H_LOC, 512], F16, tag="qcur")
                mt_start = 0
            xb8, xb8r = x_cur
            q_terms = fb_terms(wq_t, xb8, xb8r)
            if ic == 0:
                # term-major with 4 concurrent PSUM groups: the main
                # wq8*x8 term starts as soon as the first x8/wq8 DMA chunks
                # land; residual streams arrive while it runs
                ps_q = [psA.tile([P, 512], F32, tag="psA", name=f"psq{m}")
                        for m in range(2)]
                ps_q += [psV.tile([P, 512], F32, tag="psV", name=f"psq{m}")
                         for m in range(2, 4)]
                q_terms0 = ((wq_t[0], xb8), (wq_t[0], xb8r), (wq_t[1], xb8))
                for ti, (wt, xt) in enumerate(q_terms0):
                    for a in range(NP):
                        for mt in range(H_LOC):
                            nc.tensor.matmul(
                                ps_q[mt][:],
                                lhsT=wt[:, 2 * a:2 * a + 2,
                                        mt * P:(mt + 1) * P],
                                rhs=xt[:, 2 * a:2 * a + 2, :],
                                start=(a == 0 and ti == 0),
                                stop=(a == NP - 1 and ti == 2),
                                perf_mode=mybir.MatmulPerfMode.DoubleRow)
                for mt in range(H_LOC):
                    with nc.allow_low_precision(reason="fp16 q"):
                        nc.vector.tensor_scalar(
                            out=qcur[:, mt, :], in0=ps_q[mt][:],
                            scalar1=INV_S, scalar2=bq_sb[:, mt:mt + 1],
                            op0=mybir.AluOpType.mult,
                            op1=mybir.AluOpType.add)
                    rope_inplace(qcur[:, mt, :], qcur[:, mt, :], ts0,
                                 dq=nc.scalar)
            else:
                for mt in range(mt_start, H_LOC):
                    ps = psA.tile([P, 512], F32, tag="psA")
                    for ti, (wt, xt) in enumerate(q_terms):
                        for a in range(NP):
                            nc.tensor.matmul(
                                ps[:],
                                lhsT=wt[:, 2 * a:2 * a + 2,
                                        mt * P:(mt + 1) * P],
                                rhs=xt[:, 2 * a:2 * a + 2, :],
                                start=(a == 0 and ti == 0),
                                stop=(a == NP - 1 and ti == 2),
                                perf_mode=mybir.MatmulPerfMode.DoubleRow)
                    nc.scalar.activation(
                        qcur[:, mt, :], ps[:],
                        mybir.ActivationFunctionType.Identity,
                        bias=bq_sb[:, mt:mt + 1], scale=INV_S)
                    rope_inplace(qcur[:, mt, :], qcur[:, mt, :], ts0)

            def proj_k():
                if ic == 0:
                    # w-residual term last: wk8r chunks are the last arrivals
                    k_terms = ((wk_t[0], xb8), (wk_t[0], xb8r), (wk_t[1], xb8))
                else:
                    k_terms = fb_terms(wk_t, xb8, xb8r)
                for mt in range(H_LOC):
                    ps = psA.tile([P, 512], F32, tag="psA")
                    for ti, (wt, xt) in enumerate(k_terms):
                        for a in range(NP):
                            nc.tensor.matmul(
                                ps[:],
                                lhsT=wt[:, 2 * a:2 * a + 2,
                                        mt * P:(mt + 1) * P],
                                rhs=xt[:, 2 * a:2 * a + 2, :],
                                start=(a == 0 and ti == 0),
                                stop=(a == NP - 1 and ti == 2),
                                perf_mode=mybir.MatmulPerfMode.DoubleRow)
                    if ic == 0:
                        with nc.allow_low_precision(reason="fp16 k"):
                            nc.vector.tensor_scalar(
                                out=k_res[:, mt, ts0:ts0 + 512], in0=ps[:],
                                scalar1=INV_S, scalar2=bk_sb[:, mt:mt + 1],
                                op0=mybir.AluOpType.mult,
                                op1=mybir.AluOpType.add)
                    else:
                        nc.scalar.activation(
                            k_res[:, mt, ts0:ts0 + 512], ps[:],
                            mybir.ActivationFunctionType.Identity,
                            bias=bk_sb[:, mt:mt + 1], scale=INV_S)
                    rope_inplace(k_res[:, mt, ts0:ts0 + 512],
                                 k_res[:, mt, ts0:ts0 + 512], ts0,
                                 dq=nc.scalar if ic == 0 else None)

            def proj_v():
                v_terms = ((xb8, wv_t[0]), (xb8, wv_t[1]), (xb8r, wv_t[0]))
                if ic == 0:
                    # pair-major on psS (idle before attention); term-major
                    # with the w-residual term last, since the residual
                    # weight chunks are the last DMAs to land
                    ps_v = [psA.tile([P, M], F32, tag="psA", name=f"psv{t}")
                            for t in range(2)]
                    ps_v += [psV.tile([P, M], F32, tag="psV", name=f"psv{t}")
                             for t in range(2, 4)]
                    vt0 = ((xb8, wv_t[0]), (xb8r, wv_t[0]), (xb8, wv_t[1]))
                    for ti, (xt, wt) in enumerate(vt0):
                        for a in range(NP):
                            for tt in range(4):
                                nc.tensor.matmul(
                                    ps_v[tt][:],
                                    lhsT=xt[:, 2 * a:2 * a + 2,
                                            tt * P:(tt + 1) * P],
                                    rhs=wt[:, 2 * a:2 * a + 2, :],
                                    start=(a == 0 and ti == 0),
                                    stop=(a == NP - 1 and ti == 2),
                                    perf_mode=mybir.MatmulPerfMode.DoubleRow)
                    for tt in range(4):
                        nc.vector.scalar_tensor_tensor(
                            out=v_res[:, 4 * ic + tt, :], in0=ps_v[tt][:],
                            scalar=INV_S, in1=bvb_sb[:],
                            op0=mybir.AluOpType.mult, op1=mybir.AluOpType.add)
                    return
                for tt in range(4):
                    ps = psA.tile([P, M], F32, tag="psA")
                    for ti, (xt, wt) in enumerate(v_terms):
                        for a in range(NP):
                            nc.tensor.matmul(
                                ps[:],
                                lhsT=xt[:, 2 * a:2 * a + 2,
                                        tt * P:(tt + 1) * P],
                                rhs=wt[:, 2 * a:2 * a + 2, :],
                                start=(a == 0 and ti == 0),
                                stop=(a == NP - 1 and ti == 2),
                                perf_mode=mybir.MatmulPerfMode.DoubleRow)
                    nc.vector.scalar_tensor_tensor(
                        out=v_res[:, 4 * ic + tt, :], in0=ps[:],
                        scalar=INV_S, in1=bvb_sb[:],
                        op0=mybir.AluOpType.mult, op1=mybir.AluOpType.add)

            if ic == 0:
                # wk lands last on the SP queue: fill the gap with proj v
                proj_v()
                _mark(f"v{ic}")
                proj_k()
                _mark(f"k{ic}")
            else:
                proj_k()
                _mark(f"k{ic}")
                if ic == NT - 1:
                    # att3 is Act-exp-capacity-bound while the projections
                    # are Act-idle: pull head-0's first off-diag score pairs
                    # forward so their exps overlap the v projection
                    for pj in range(4):
                        jt0, jt1 = 2 * pj, 2 * pj + 1
                        ps2p = psS.tile([P, 2, 512], F32, tag="psS2")
                        nc.tensor.matmul(
                            ps2p[:, 0, :],
                            lhsT=k_res[:, 0, jt0 * P:(jt0 + 1) * P],
                            rhs=qcur[:, 0, :], start=True, stop=True)
                        nc.tensor.matmul(
                            ps2p[:, 1, :],
                            lhsT=k_res[:, 0, jt1 * P:(jt1 + 1) * P],
                            rhs=qcur[:, 0, :], start=True, stop=True)
                        ex2p = expool.tile([P, 2, 512], F16, tag="ex")
                        nc.scalar.activation(
                            ex2p[:, :, :], ps2p[:, :, :],
                            mybir.ActivationFunctionType.Exp,
                            bias=eb_sb[:, 0:1], scale=SCALE)
                        pre_ex[pj] = ex2p
                proj_v()
                _mark(f"v{ic}")

            if ic + 1 < NT:
                x_next = load_x(ic + 1)

            if ic == 0:
                # wo is first needed by ph3(0) woven into attention(1);
                # dispatch after x1 so it doesn't steal DMA bandwidth from
                # the wv/wk/x1 loads the pipeline stalls on. Sync queue: the
                # scalar queue is head-of-line blocked on chunk-0 rope
                # shuffles; the gpsimd SWDGE path trickles.
                for j in range(H_LOC):
                    nc.sync.dma_start(out=wo_b[:, j:j + 1, :],
                                      in_=wor[:, j:j + 1, :])
                    nc.sync.dma_start(out=wor_b[:, j:j + 1, :],
                                      in_=worr[:, j:j + 1, :])

            # ---- attention for chunk ic (weaving ph3 of chunk ic-1) ----
            attn8 = atpool.tile([P, H_LOC, 512], F8, tag="attn8")
            attn8r = atpool.tile([P, H_LOC, 512], F8, tag="attn8r")
            njt = 4 * ic + 4
            slots_left = H_LOC * njt

            def emit_chain(dfr):
                """Deferred per-head softmax-normalize chain: recip + fp8
                attn split. Deferred into the NEXT head's tile loop so the
                in-order DVE queue never parks waiting for the Pool
                all_reduce (which sits behind woven ph3 evicts)."""
                rden_d, ps_av_d, hh = dfr
                with nc.allow_low_precision(reason="softmax reciprocal"):
                    if rden_d.space == tile.bass.MemorySpace.PSUM:
                        den_ps = rden_d
                        rden_d = rdpool.tile([P, 512], F16, tag="rden")
                        nc.vector.reciprocal(rden_d[:], den_ps[:, 0, :])
                    else:
                        nc.vector.reciprocal(rden_d[:], rden_d[:])
                    nc.vector.scalar_tensor_tensor(
                        out=attn8[:, hh, :], in0=ps_av_d[:], scalar=SA,
                        in1=rden_d[:],
                        op0=mybir.AluOpType.mult, op1=mybir.AluOpType.mult)
                    a16 = a16pool.tile([P, 512], F16, tag="a16")
                    nc.vector.scalar_tensor_tensor(
                        out=a16[:], in0=ps_av_d[:], scalar=SA, in1=rden_d[:],
                        op0=mybir.AluOpType.mult, op1=mybir.AluOpType.mult)
                    eng = nc.vector if hh == H_LOC - 1 else nc.gpsimd
                    eng.tensor_tensor(
                        attn8r[:, hh, :], a16[:], attn8[:, hh, :],
                        mybir.AluOpType.subtract)

            deferred = None
            ic0_chains = []
            n_items = CT * 6
            slots_done = 0
            for h in range(H_LOC):
                ps_av = psV.tile([P, 512], F32, tag="psV")
                exsum = espool.tile([P, 512], F16, tag="exsum")
                prev = None  # (ex2 tile, c00, c01, jt0) awaiting av matmuls
                for pj in range(njt // 2):
                    jt0, jt1 = 2 * pj, 2 * pj + 1
                    d0, d1 = jt0 - 4 * ic, jt1 - 4 * ic
                    c00 = 128 * d0 if d0 > 0 else 0
                    c01 = 128 * d1 if d1 > 0 else 0
                    if pj == min(2, njt // 2 - 1) and deferred is not None:
                        emit_chain(deferred)
                        deferred = None
                    if h == 0 and pj in pre_ex:
                        # precomputed during the projection phase
                        ex2 = pre_ex[pj]
                        slots_done += 2
                        if pending is not None:
                            eff = slots_left + (36 if ic == NT - 1 else 0)
                            tgt = min(n_items,
                                      (n_items * slots_done) // eff)
                            pending.step(tgt - pending.pos)
                        if prev is not None:
                            pex2, pc00, pc01, pjt0 = prev
                            nc.tensor.matmul(
                                ps_av[:, pc00:],
                                lhsT=v_res[:, pjt0, h * HS:(h + 1) * HS],
                                rhs=pex2[:, 0, pc00:],
                                start=(pjt0 == 0), stop=False,
                                skip_group_check=True)
                            nc.tensor.matmul(
                                ps_av[:, pc01:],
                                lhsT=v_res[:, pjt0 + 1, h * HS:(h + 1) * HS],
                                rhs=pex2[:, 1, pc01:],
                                start=False, stop=False,
                                skip_group_check=True)
                        with nc.allow_low_precision(reason="fp16 denom"):
                            if pj == 0:
                                nc.vector.tensor_tensor(
                                    exsum[:], ex2[:, 0, :], ex2[:, 1, :],
                                    mybir.AluOpType.add)
                            else:
                                nc.vector.tensor_tensor(
                                    exsum[:], exsum[:],
                                    ex2[:, 0, :], mybir.AluOpType.add)
                                nc.vector.tensor_tensor(
                                    exsum[:], exsum[:],
                                    ex2[:, 1, :], mybir.AluOpType.add)
                        prev = (ex2, 0, 0, jt0)
                        continue
                    # scores for a PAIR of key tiles into one 2-bank PSUM
                    # tile; one Act exp over both planes amortizes the
                    # fixed per-op Act overhead (the attention pacer)
                    ps2 = psS.tile([P, 2, 512], F32, tag="psS2")
                    nc.tensor.matmul(
                        ps2[:, 0, c00:],
                        lhsT=k_res[:, h, jt0 * P:(jt0 + 1) * P],
                        rhs=qcur[:, h, c00:],
                        start=True, stop=True)
                    split = d0 >= 0 and ic < NT - 1
                    # diagonal pairs in PE-bound windows: exact-trim plane 1
                    # and pay a second (smaller) exp on the slack Act engine.
                    # Otherwise plane 1 also starts at c00: cols [c00:c01]
                    # are real (causal-masked) scores so the shared exp
                    # never reads uninitialized PSUM
                    c1lo = c01 if split else c00
                    nc.tensor.matmul(
                        ps2[:, 1, c1lo:],
                        lhsT=k_res[:, h, jt1 * P:(jt1 + 1) * P],
                        rhs=qcur[:, h, c1lo:],
                        start=True, stop=True)
                    ex2 = expool.tile([P, 2, 512], F16, tag="ex")
                    if split:
                        nc.scalar.activation(
                            ex2[:, 0, c00:], ps2[:, 0, c00:],
                            mybir.ActivationFunctionType.Exp,
                            bias=eb_sb[:, 0:1], scale=SCALE)
                        nc.scalar.activation(
                            ex2[:, 1, c01:], ps2[:, 1, c01:],
                            mybir.ActivationFunctionType.Exp,
                            bias=eb_sb[:, 0:1], scale=SCALE)
                    else:
                        nc.scalar.activation(
                            ex2[:, :, c00:], ps2[:, :, c00:],
                            mybir.ActivationFunctionType.Exp,
                            bias=eb_sb[:, 0:1], scale=SCALE)
                    # causal triangle: for every diagonal tile the global
                    # query base ts0+c0 equals the key base jt*P, so one
                    # [P,P] keep-where-col>=row mask serves them all
                    if d0 >= 0:
                        nc.vector.tensor_tensor(
                            ex2[:, 0, c00:c00 + P], ex2[:, 0, c00:c00 + P],
                            tri_sb[:], mybir.AluOpType.mult)
                    if d1 >= 0:
                        nc.vector.tensor_tensor(
                            ex2[:, 1, c01:c01 + P], ex2[:, 1, c01:c01 + P],
                            tri_sb[:], mybir.AluOpType.mult)
                    slots_done += 2
                    if pending is not None:
                        # spread ph3 items evenly across the window; on the
                        # last chunk hold ~16 back so they bridge the final
                        # head's normalize-chain latency after the window
                        eff = slots_left + (36 if ic == NT - 1 else 0)
                        tgt = min(n_items, (n_items * slots_done) // eff)
                        pending.step(tgt - pending.pos)
                    elif qw is not None:
                        nqw = len(qw.items)
                        qw.step((nqw * slots_done) // slots_left - qw.pos)
                    if prev is not None:
                        pex2, pc00, pc01, pjt0 = prev
                        nc.tensor.matmul(
                            ps_av[:, pc00:],
                            lhsT=v_res[:, pjt0, h * HS:(h + 1) * HS],
                            rhs=pex2[:, 0, pc00:],
                            start=(pjt0 == 0), stop=False,
                            skip_group_check=True)
                        nc.tensor.matmul(
                            ps_av[:, pc01:],
                            lhsT=v_res[:, pjt0 + 1, h * HS:(h + 1) * HS],
                            rhs=pex2[:, 1, pc01:],
                            start=False, stop=False,
                            skip_group_check=True)
                    with nc.allow_low_precision(reason="fp16 softmax denom"):
                        if pj == 0:
                            if ic == 0:
                                nc.vector.tensor_copy(out=exsum[:],
                                                      in_=ex2[:, 0, :])
                                nc.vector.tensor_tensor(
                                    exsum[:, c01:], exsum[:, c01:],
                                    ex2[:, 1, c01:], mybir.AluOpType.add)
                            else:
                                nc.vector.tensor_tensor(
                                    exsum[:], ex2[:, 0, :], ex2[:, 1, :],
                                    mybir.AluOpType.add)
                        else:
                            nc.vector.tensor_tensor(
                                exsum[:, c00:], exsum[:, c00:],
                                ex2[:, 0, c00:], mybir.AluOpType.add)
                            nc.vector.tensor_tensor(
                                exsum[:, c01:], exsum[:, c01:],
                                ex2[:, 1, c01:], mybir.AluOpType.add)
                    prev = (ex2, c00, c01, jt0)
                pex2, pc00, pc01, pjt0 = prev
                nc.tensor.matmul(
                    ps_av[:, pc00:],
                    lhsT=v_res[:, pjt0, h * HS:(h + 1) * HS],
                    rhs=pex2[:, 0, pc00:],
                    start=(pjt0 == 0), stop=False,
                    skip_group_check=True)
                nc.tensor.matmul(
                    ps_av[:, pc01:],
                    lhsT=v_res[:, pjt0 + 1, h * HS:(h + 1) * HS],
                    rhs=pex2[:, 1, pc01:],
                    start=False, stop=True,
                    skip_group_check=True)
                if (h == H_LOC - 1 and ic == NT - 1) or ic == 0:
                    # chunk 0 (PE idle, DVE-bound) and end of kernel (PE
                    # free, chain gates the output projection): sum the
                    # denominator with a ones-matmul instead of the slower
                    # Pool all_reduce
                    ps_d = psS.tile([P, 2, 512], F32, tag="psS2")
                    nc.tensor.matmul(
                        ps_d[:, 0, :], lhsT=ones_sb[:], rhs=exsum[:],
                        start=True, stop=True)
                    deferred = (ps_d, ps_av, h)
                else:
                    rden = rdpool.tile([P, 512], F16, tag="rden")
                    nc.gpsimd.partition_all_reduce(
                        rden[:], exsum[:], channels=P,
                        reduce_op=bass_isa.ReduceOp.add)
                    deferred = (rden, ps_av, h)
                if h == H_LOC - 1 and ic < NT - 1:
                    emit_chain(deferred)
                    deferred = None

            _mark(f"att{ic}")
            for dfr in ic0_chains:
                emit_chain(dfr)
            if pending is not None:
                pending.finish()
            _mark(f"ph3fin{ic}")
            pending = Ph3(ic, attn8, attn8r,
                          pools=[psA, psV] if ic == NT - 1 else None)
            if deferred is not None:
                # last chunk: co0's three g0 steps only read heads 0-1; emit
                # them BEFORE the last head's normalize chain so they don't
                # inherit a wait on its attn8 write (sem counts are
                # emission-order conservative) and bridge its latency
                pending.step(6)
                emit_chain(deferred)
                deferred = None

        pending.finish()
        _mark("ph3last")

    nc.finalize()
    return nc


def get_nc(phases=(1, 2, 3)):
    if phases not in _NC_CACHE:
        _NC_CACHE[phases] = _build(phases)
    return _NC_CACHE[phases]


def _rope_tables():
    inv_freq = 1.0 / (BASE ** (np.arange(0, ROT, 2, dtype=np.float64) / ROT))
    freqs = np.arange(T, dtype=np.float64)[:, None] * inv_freq[None, :]  # [T, 32]
    cos_h = np.cos(freqs).T.astype(np.float32)   # [32, T]
    sin_h = np.sin(freqs).T.astype(np.float32)
    cosT = np.concatenate([cos_h, cos_h], axis=0)          # [64, T]
    nsT = np.concatenate([-sin_h, sin_h], axis=0)          # [64, T] signed sin
    return (np.ascontiguousarray(cosT.astype(np.float16)),
            np.ascontiguousarray(nsT.astype(np.float16)))


def _split8(a, s):
    """a*s = a8 + a8r (both fp8 e4m3) up to second-order quantization."""
    scaled = a * np.float32(s)
    a8 = scaled.astype(NPF8)
    a8r = (scaled - a8.astype(np.float32)).astype(NPF8)
    return np.ascontiguousarray(a8), np.ascontiguousarray(a8r)


def make_in_maps(x, Wq, bq, Wk, bk, Wv, bv, Wo, bo):
    cosT, nsT = _rope_tables()
    in_maps = []
    for c in range(N_CORES):
        b, g = divmod(c, TPG)
        ms = slice(g * M, (g + 1) * M)
        xh8, xh8r = _split8(x[b].T, SX)
        wq8, wq8r = _split8(Wq[ms].T, SW)
        wk8, wk8r = _split8(Wk[ms].T, SW)
        wv8, wv8r = _split8(Wv[ms].T, SW)
        wo8, wo8r = _split8(Wo[:, ms].T, SWO)
        in_maps.append({
            "xh8": xh8, "xh8r": xh8r,
            "wq8": wq8, "wq8r": wq8r,
            "wk8": wk8, "wk8r": wk8r,
            "wv8": wv8, "wv8r": wv8r,
            "wo8T": wo8, "wo8rT": wo8r,
            "bqc": np.ascontiguousarray(bq[ms].reshape(H_LOC, P).T),
            "bkc": np.ascontiguousarray(bk[ms].reshape(H_LOC, P).T),
            "bvr": np.ascontiguousarray(bv[ms].reshape(1, M).astype(np.float16)),
            "ebias": np.full((P, 1), EXP_BIAS, np.float32),
            "trimask": np.triu(np.ones((P, P), np.float16)),
            "cosT": cosT,
            "nsT": nsT,
        })
    return in_maps


def assemble(results, bo):
    out = np.empty((B, T, C), dtype=np.float32)
    for b in range(B):
        acc = results[b * TPG]["outT"].astype(np.float32)
        for g in range(1, TPG):
            acc = acc + results[b * TPG + g]["outT"].astype(np.float32)
        out[b] = acc.T + bo[None, :]
    return out


def kernel(x, Wq, bq, Wk, bk, Wv, bv, Wo, bo):
    nc = get_nc()
    in_maps = make_in_maps(np.asarray(x, np.float32),
                           np.asarray(Wq, np.float32), np.asarray(bq, np.float32),
                           np.asarray(Wk, np.float32), np.asarray(bk, np.float32),
                           np.asarray(Wv, np.float32), np.asarray(bv, np.float32),
                           np.asarray(Wo, np.float32), np.asarray(bo, np.float32))
    res = run_bass_kernel_spmd(nc, in_maps, list(range(N_CORES)))
    return assemble(res.results, np.asarray(bo, np.float32))



# revision 98
# speedup vs baseline: 1.0089x; 1.0017x over previous
"""Trainium2 Bass kernel for a full causal MHA layer (B=2, T=2048, C=2048, H=16,
partial RoPE on first 64 dims of each 128-dim head).

Sharding over 8 cores: core c handles batch b=c//4 and heads [4g, 4g+4), g=c%4
(tensor-parallel over heads x data-parallel over batch).

Fully fused single pass per core, fp16 data plane (fp32 PSUM accumulation):
  for each 512-token chunk ic:
    proj q/k (fp16 weights stationary, fp16 x moving), bias+partial-RoPE,
      q and k stay resident in SBUF (no DRAM spills)
    proj v -> v_res [key, jt, m] fp16 resident
    attention for chunk ic over heads h: per key-tile jt
      scoresT[k,q] (k_res stationary fp16, q moving fp16)
      -> exp(scale*s - 10*ln2) -> ex fp16 (Act), triangle mask on diagonal
         tiles only (DVE mult by a const [128,128] triu mask), exact causal
         col-trimming
      -> av accumulation outT[d,q] via PE; softmax denominator via DVE
         exsum adds + gpsimd partition_all_reduce (no PE ones-matmuls)
      output-projection matmuls of chunk ic-1 are woven between attention
      matmuls to keep PE busy during Act-latency windows
    phase3(ic): out partial outT[c,q] = sum_mt woT attn, DVE evict fp16, DMA
Host: slices inputs per core (fp16), sums the 4 TP partials per batch + bo.
"""

import math

import ml_dtypes
import numpy as np

NPF8 = ml_dtypes.float8_e4m3

import concourse.bass_isa as bass_isa
import concourse.mybir as mybir
import concourse.tile as tile
from concourse import bacc
from concourse.bass_utils import run_bass_kernel_spmd

F32 = mybir.dt.float32
F16 = mybir.dt.float16
F8 = mybir.dt.float8e4

B, T, C = 2, 2048, 2048
H = 16
HS = 128
ROT = 64
HALF = 32
BASE = 10000.0

N_CORES = 8
TPG = 4                # TP group size (heads split)
H_LOC = H // TPG       # 4 heads per core
M = H_LOC * HS         # 512 local head-dim columns
SCALE = 1.0 / math.sqrt(HS)
EXP_BIAS = -10.0 * math.log(2.0)   # exp(s*SCALE - 10ln2): keeps fp16 in range
SX = 16.0                 # fp8 quantization scale for x
SW = 1024.0               # fp8 quantization scale for Wq/Wk/Wv
INV_S = 1.0 / (SX * SW)   # folded into the projection evictions
SWO = 1024.0              # fp8 quantization scale for Wo
SA = 64.0                 # fp8 quantization scale for attn outputs
INV_SO = 1.0 / (SWO * SA)  # folded into the output-proj evictions

P = 128
NT = T // 512          # 4 t-chunks of 512
CT = C // P            # 16 contraction tiles
JT = T // P            # 16 key tiles per head

_NC_CACHE = {}
PHASE_MARKS = []  # (label, last-emitted instruction name); debug aid only


def _build(phases=(1, 2, 3)):
    nc = bacc.Bacc(None, target_bir_lowering=False)
    PHASE_MARKS.clear()

    def _mark(label):
        # consumes one instruction name as a monotonic position marker
        PHASE_MARKS.append((label, nc.get_next_instruction_name()))

    xh8 = nc.declare_dram_parameter("xh8", [C, T], F8, isOutput=False)
    xh8r = nc.declare_dram_parameter("xh8r", [C, T], F8, isOutput=False)
    wq8 = nc.declare_dram_parameter("wq8", [C, M], F8, isOutput=False)
    wq8r = nc.declare_dram_parameter("wq8r", [C, M], F8, isOutput=False)
    wk8 = nc.declare_dram_parameter("wk8", [C, M], F8, isOutput=False)
    wk8r = nc.declare_dram_parameter("wk8r", [C, M], F8, isOutput=False)
    wv8 = nc.declare_dram_parameter("wv8", [C, M], F8, isOutput=False)
    wv8r = nc.declare_dram_parameter("wv8r", [C, M], F8, isOutput=False)
    wo8T = nc.declare_dram_parameter("wo8T", [M, C], F8, isOutput=False)
    wo8rT = nc.declare_dram_parameter("wo8rT", [M, C], F8, isOutput=False)
    bqc = nc.declare_dram_parameter("bqc", [P, H_LOC], F32, isOutput=False)
    bkc = nc.declare_dram_parameter("bkc", [P, H_LOC], F32, isOutput=False)
    bvr = nc.declare_dram_parameter("bvr", [1, M], F16, isOutput=False)
    ebias = nc.declare_dram_parameter("ebias", [P, 1], F32, isOutput=False)
    trimask = nc.declare_dram_parameter("trimask", [P, P], F16, isOutput=False)
    cosT = nc.declare_dram_parameter("cosT", [ROT, T], F16, isOutput=False)
    nsT = nc.declare_dram_parameter("nsT", [ROT, T], F16, isOutput=False)
    outT = nc.declare_dram_parameter("outT", [C, T], F16, isOutput=True)

    with tile.TileContext(nc) as tc, \
         tc.tile_pool(name="const", bufs=1) as const, \
         tc.tile_pool(name="xp", bufs=CT) as xpool, \
         tc.tile_pool(name="qc", bufs=2) as qpool, \
         tc.tile_pool(name="at", bufs=2) as atpool, \
         tc.tile_pool(name="rp", bufs=6) as rpool, \
         tc.tile_pool(name="exp", bufs=5) as expool, \
         tc.tile_pool(name="exs", bufs=3) as espool, \
         tc.tile_pool(name="rd", bufs=5) as rdpool, \
         tc.tile_pool(name="a16", bufs=2) as a16pool, \
         tc.tile_pool(name="oe", bufs=3) as oepool, \
         tc.tile_pool(name="psA", bufs=2, space="PSUM") as psA, \
         tc.tile_pool(name="psS", bufs=2, space="PSUM") as psS, \
         tc.tile_pool(name="psV", bufs=2, space="PSUM") as psV:

        cos_sb = const.tile([ROT, T], F16, tag="cos")
        ns_sb = const.tile([ROT, T], F16, tag="ns")
        bq_sb = const.tile([P, H_LOC], F32, tag="bq")
        bk_sb = const.tile([P, H_LOC], F32, tag="bk")
        bvb_sb = const.tile([P, M], F16, tag="bvb")
        eb_sb = const.tile([P, 1], F32, tag="ebias")
        tri_sb = const.tile([P, P], F16, tag="trimask")
        k_res = const.tile([P, H_LOC, T], F16, tag="kres")
        v_res = const.tile([P, JT, M], F16, tag="vres")
        wq_t = [const.tile([P, CT, M], F8, tag=f"wq{i}", name=f"wq{i}")
                for i in range(2)]
        wk_t = [const.tile([P, CT, M], F8, tag=f"wk{i}", name=f"wk{i}")
                for i in range(2)]
        wv_t = [const.tile([P, CT, M], F8, tag=f"wv{i}", name=f"wv{i}")
                for i in range(2)]
        wo_b = const.tile([P, H_LOC, C], F8, tag="wob")
        wor_b = const.tile([P, H_LOC, C], F8, tag="worb")
        wre = [d[:].rearrange("(ct p) m -> p ct m", p=P)
               for d in (wq8, wq8r, wk8, wk8r, wv8, wv8r)]
        wqre, wqrre, wkre, wkrre, wvre, wvrre = wre
        wor = wo8T[:].rearrange("(mt p) c -> p mt c", p=P)
        worr = wo8rT[:].rearrange("(mt p) c -> p mt c", p=P)

        xr = xh8[:].rearrange("(ct p) t -> p ct t", p=P)
        xrr = xh8r[:].rearrange("(ct p) t -> p ct t", p=P)
        otr = outT[:].rearrange("(co p) t -> p co t", p=P)

        def load_x(ic):
            ts0 = ic * 512
            xb = xpool.tile([P, CT, 512], F8, tag="xb8", name=f"xb{ic}", bufs=2)
            xbr = xpool.tile([P, CT, 512], F8, tag="xb8r", name=f"xbr{ic}",
                             bufs=2)
            for j in range(4):
                nc.sync.dma_start(out=xb[:, 4 * j:4 * j + 4, :],
                                  in_=xr[:, 4 * j:4 * j + 4, ts0:ts0 + 512])
                nc.sync.dma_start(out=xbr[:, 4 * j:4 * j + 4, :],
                                  in_=xrr[:, 4 * j:4 * j + 4, ts0:ts0 + 512])
            return xb, xbr

        # startup, ordered by first use: main-term operands (x8 + wq8) first,
        # then the residual streams, then wv (main, res), wk (main, res).
        # x on the SP queue, wq on the Act queue so the two interleave on the
        # DMA engines; chunked so HWDGE overheads don't pace it.
        xb0 = xpool.tile([P, CT, 512], F8, tag="xb8", name="xb0", bufs=2)
        xb0r = xpool.tile([P, CT, 512], F8, tag="xb8r", name="xb0r", bufs=2)
        # first ct-pair as its own small DMA so the very first matmul can
        # start as early as possible
        nc.sync.dma_start(out=xb0[:, 0:2, :], in_=xr[:, 0:2, 0:512])
        nc.scalar.dma_start(out=wq_t[0][:, 0:2, :], in_=wqre[:, 0:2, :])
        nc.sync.dma_start(out=xb0[:, 2:4, :], in_=xr[:, 2:4, 0:512])
        nc.scalar.dma_start(out=wq_t[0][:, 2:4, :], in_=wqre[:, 2:4, :])
        for j in range(1, 4):
            a, b = 4 * j, 4 * j + 4
            nc.sync.dma_start(out=xb0[:, a:b, :], in_=xr[:, a:b, 0:512])
            nc.scalar.dma_start(out=wq_t[0][:, a:b, :], in_=wqre[:, a:b, :])
        for j in range(4):
            a, b = 4 * j, 4 * j + 4
            nc.sync.dma_start(out=xb0r[:, a:b, :], in_=xrr[:, a:b, 0:512])
            nc.scalar.dma_start(out=wq_t[1][:, a:b, :], in_=wqrre[:, a:b, :])
        for j in range(4):
            a, b = 4 * j, 4 * j + 4
            nc.sync.dma_start(out=wv_t[0][:, a:b, :], in_=wvre[:, a:b, :])
        for j in range(4):
            a, b = 4 * j, 4 * j + 4
            nc.sync.dma_start(out=wv_t[1][:, a:b, :], in_=wvrre[:, a:b, :])
        for j in range(4):
            a, b = 4 * j, 4 * j + 4
            nc.sync.dma_start(out=wk_t[0][:, a:b, :], in_=wkre[:, a:b, :])
        for j in range(4):
            a, b = 4 * j, 4 * j + 4
            nc.sync.dma_start(out=wk_t[1][:, a:b, :], in_=wkrre[:, a:b, :])
        ones_sb = const.tile([P, P], F16, tag="ones")
        nc.gpsimd.memset(ones_sb[:], 1.0)
        nc.gpsimd.dma_start(out=bq_sb[:], in_=bqc[:])
        nc.gpsimd.dma_start(out=bk_sb[:], in_=bkc[:])
        nc.gpsimd.dma_start(out=cos_sb[:], in_=cosT[:])
        nc.gpsimd.dma_start(out=ns_sb[:], in_=nsT[:])
        nc.gpsimd.dma_start(out=eb_sb[:], in_=ebias[:])
        nc.gpsimd.dma_start(out=tri_sb[:], in_=trimask[:])
        nc.gpsimd.dma_start(out=bvb_sb[:], in_=bvr[0:1, :].to_broadcast([P, M]))

        NP = CT // 2   # 8 ct-pairs per contraction

        def fb_terms(w_t, xb, xbr):
            """(lhsT_tile, rhs_tile) per error-feedback term: main, w-res,
            x-res. All DoubleRow fp8 over ct-pairs."""
            return ((w_t[0], xb), (w_t[1], xb), (w_t[0], xbr))

        def rope_inplace(dst, tmp_src, ts0, dq=None):
            """dst[0:ROT, 512] fp16 <- rope(tmp_src rows 0:ROT) in place.
            tmp_src rows are pre-rope biased values; dst may alias tmp_src.
            dq picks the DMA queue for the partition-swap (the SP queue is
            backed up with weight loads during chunk 0)."""
            dq = dq or nc.sync
            sh = rpool.tile([ROT, 512], F16, tag="sh")
            dq.dma_start(out=sh[0:HALF], in_=tmp_src[HALF:ROT])
            dq.dma_start(out=sh[HALF:ROT], in_=tmp_src[0:HALF])
            rot = rpool.tile([ROT, 512], F16, tag="rot")
            nc.vector.tensor_tensor(rot[:], sh[:], ns_sb[:, ts0:ts0 + 512],
                                    mybir.AluOpType.mult)
            tcos = rpool.tile([ROT, 512], F16, tag="tcos")
            nc.vector.tensor_tensor(tcos[:], tmp_src[:ROT], cos_sb[:, ts0:ts0 + 512],
                                    mybir.AluOpType.mult)
            nc.vector.tensor_tensor(dst[0:ROT], tcos[:], rot[:],
                                    mybir.AluOpType.add)

        class Ph3:
            """Output projection for chunk ic (fp8 DoubleRow, 3-term error
            feedback); matmuls are dispensed one at a time (step) so they
            weave between attention matmuls. 6 steps per co:
            (term, g) with term in {wo8*a8, wo8r*a8, wo8*a8r}, g the DR pair
            of mt planes."""

            def __init__(self, ic, a8, a8r, pools=None):
                self.ic = ic
                self.terms = ((wo_b, a8), (wor_b, a8), (wo_b, a8r))
                self.items = [(co, s) for co in range(CT) for s in range(6)]
                if ic == NT - 1:
                    # stagger co0/co1: their g0 steps (heads 0-1 only) first,
                    # g1 steps after — bridges the last head's normalize
                    # chain at the attention/finish boundary
                    self.items = (
                        [(0, 0), (0, 1), (0, 2), (1, 0), (1, 1), (1, 2),
                         (0, 3), (0, 4), (0, 5), (1, 3), (1, 4), (1, 5)]
                        + [(co, s) for co in range(2, CT) for s in range(6)])
                self.pos = 0
                self.ps = None
                self.pools = pools or [psA]
                self.finishing = False

            def step(self, n=1):
                for _ in range(n):
                    if self.pos >= len(self.items):
                        return
                    co, s = self.items[self.pos]
                    self.pos += 1
                    # g-major: the three g=0 steps only read heads 0-1,
                    # which are normalized well before heads 2-3 land
                    g, t = divmod(s, 3)
                    if s == 0:
                        pool = self.pools[co % len(self.pools)]
                        ps = pool.tile([P, 512], F32,
                                       tag="psA" if pool is psA else "psV",
                                       name=f"ps3_{self.ic}_{co}")
                        self.ps_by_co = getattr(self, "ps_by_co", {})
                        self.ps_by_co[co] = ps
                    self.ps = self.ps_by_co[co]
                    wt, at = self.terms[t]
                    nc.tensor.matmul(
                        self.ps[:],
                        lhsT=wt[:, 2 * g:2 * g + 2, co * P:(co + 1) * P],
                        rhs=at[:, 2 * g:2 * g + 2, :],
                        start=(s == 0), stop=(s == 5),
                        perf_mode=mybir.MatmulPerfMode.DoubleRow)
                    if s == 5:
                        if co % 4 == 0:
                            self.ot = oepool.tile([P, 4, 512], F16, tag="ot")
                        last = self.ic == NT - 1
                        if self.finishing and last and co % 2 == 0:
                            # end of kernel: DVE is idle, alternate with Act
                            # so the final evict drain is 2-wide
                            with nc.allow_low_precision(reason="fp16 out"):
                                nc.vector.tensor_scalar(
                                    out=self.ot[:, co % 4, :], in0=self.ps[:],
                                    scalar1=INV_SO, scalar2=None,
                                    op0=mybir.AluOpType.mult)
                        elif self.finishing:
                            # post-attention block: DVE is draining attention
                            # tail work, evict on Act
                            nc.scalar.activation(
                                self.ot[:, co % 4, :], self.ps[:],
                                mybir.ActivationFunctionType.Identity,
                                scale=INV_SO)
                        elif co % 3 == 2:
                            nc.scalar.activation(
                                self.ot[:, co % 4, :], self.ps[:],
                                mybir.ActivationFunctionType.Identity,
                                scale=INV_SO)
                        else:
                            # woven between attention matmuls: mostly DVE
                            # with Act picking up every third (gpsimd can't
                            # read PSUM)
                            with nc.allow_low_precision(reason="fp16 out"):
                                nc.vector.tensor_scalar(
                                    out=self.ot[:, co % 4, :], in0=self.ps[:],
                                    scalar1=INV_SO, scalar2=None,
                                    op0=mybir.AluOpType.mult)
                        # last chunk: per-co DMAs on alternating queues so
                        # the post-matmul tail is one small empty-pipe
                        # transfer, not a backlog of big ones
                        step = (1 if co >= 12 else 2) if last else 2
                        if co % step == step - 1:
                            j0 = co % 4 - (step - 1)
                            # co14/15 go on sync: its DMA dispatch is half
                            # the scalar queue's, and it's empty by then
                            dq = (nc.sync if co >= 14 else nc.scalar) \
                                if (last and co % 2 == 1) else nc.sync
                            dq.dma_start(
                                out=otr[:, co - step + 1:co + 1,
                                        self.ic * 512:self.ic * 512 + 512],
                                in_=self.ot[:, j0:j0 + step, :])

            def finish(self):
                self.finishing = True
                self.step(len(self.items) - self.pos)

        class Qweave:
            """q-projection of chunk 1, head-tiles mt0-mt1, woven a few
            matmuls at a time into attention(0)'s gaps (no ph3 exists yet,
            and attention(0) is DVE-chain-bound)."""

            N_MT = 2

            def __init__(self, xb1, xb1r):
                self.qcur = qpool.tile([P, H_LOC, 512], F16, tag="qcur",
                                       name="qcur1")
                self.terms = ((wq_t[0], xb1), (wq_t[1], xb1), (wq_t[0], xb1r))
                self.items = [(mt, ti, a) for mt in range(self.N_MT)
                              for ti in range(3) for a in range(NP)]
                self.pos = 0
                self.ps = None

            def step(self, n=1):
                for _ in range(n):
                    if self.pos >= len(self.items):
                        return
                    mt, ti, a = self.items[self.pos]
                    self.pos += 1
                    if ti == 0 and a == 0:
                        self.ps = psA.tile([P, 512], F32, tag="psA",
                                           name=f"psqw{mt}")
                    wt, xt = self.terms[ti]
                    nc.tensor.matmul(
                        self.ps[:],
                        lhsT=wt[:, 2 * a:2 * a + 2, mt * P:(mt + 1) * P],
                        rhs=xt[:, 2 * a:2 * a + 2, :],
                        start=(ti == 0 and a == 0),
                        stop=(ti == 2 and a == NP - 1),
                        perf_mode=mybir.MatmulPerfMode.DoubleRow)
                    if ti == 2 and a == NP - 1:
                        nc.scalar.activation(
                            self.qcur[:, mt, :], self.ps[:],
                            mybir.ActivationFunctionType.Identity,
                            bias=bq_sb[:, mt:mt + 1], scale=INV_S)
                        rope_inplace(self.qcur[:, mt, :],
                                     self.qcur[:, mt, :], 512)

        pending = None
        qw = None

        for ic in range(NT):
            ts0 = ic * 512
            x_cur = (xb0, xb0r) if ic == 0 else x_next

            # ---- proj q ----
            _mark(f"pre_q{ic}")
            pre_ex = {}
            if ic == 1 and qw is not None:
                qcur = qw.qcur    # first tiles computed during attention(0)
                mt_start = qw.N_MT
            else:
                qcur = qpool.tile([P, H_LOC, 512], F16, tag="qcur")
                mt_start = 0
            xb8, xb8r = x_cur
            q_terms = fb_terms(wq_t, xb8, xb8r)
            if ic == 0:
                # term-major with 4 concurrent PSUM groups: the main
                # wq8*x8 term starts as soon as the first x8/wq8 DMA chunks
                # land; residual streams arrive while it runs
                ps_q = [psA.tile([P, 512], F32, tag="psA", name=f"psq{m}")
                        for m in range(2)]
                ps_q += [psV.tile([P, 512], F32, tag="psV", name=f"psq{m}")
                         for m in range(2, 4)]
                q_terms0 = ((wq_t[0], xb8), (wq_t[0], xb8r), (wq_t[1], xb8))
                for ti, (wt, xt) in enumerate(q_terms0):
                    for a in range(NP):
                        for mt in range(H_LOC):
                            nc.tensor.matmul(
                                ps_q[mt][:],
                                lhsT=wt[:, 2 * a:2 * a + 2,
                                        mt * P:(mt + 1) * P],
                                rhs=xt[:, 2 * a:2 * a + 2, :],
                                start=(a == 0 and ti == 0),
                                stop=(a == NP - 1 and ti == 2),
                                perf_mode=mybir.MatmulPerfMode.DoubleRow)
                for mt in range(H_LOC):
                    with nc.allow_low_precision(reason="fp16 q"):
                        nc.vector.tensor_scalar(
                            out=qcur[:, mt, :], in0=ps_q[mt][:],
                            scalar1=INV_S, scalar2=bq_sb[:, mt:mt + 1],
                            op0=mybir.AluOpType.mult,
                            op1=mybir.AluOpType.add)
                    rope_inplace(qcur[:, mt, :], qcur[:, mt, :], ts0,
                                 dq=nc.scalar)
            else:
                for mt in range(mt_start, H_LOC):
                    ps = psA.tile([P, 512], F32, tag="psA")
                    for ti, (wt, xt) in enumerate(q_terms):
                        for a in range(NP):
                            nc.tensor.matmul(
                                ps[:],
                                lhsT=wt[:, 2 * a:2 * a + 2,
                                        mt * P:(mt + 1) * P],
                                rhs=xt[:, 2 * a:2 * a + 2, :],
                                start=(a == 0 and ti == 0),
                                stop=(a == NP - 1 and ti == 2),
                                perf_mode=mybir.MatmulPerfMode.DoubleRow)
                    nc.scalar.activation(
                        qcur[:, mt, :], ps[:],
                        mybir.ActivationFunctionType.Identity,
                        bias=bq_sb[:, mt:mt + 1], scale=INV_S)
                    rope_inplace(qcur[:, mt, :], qcur[:, mt, :], ts0)

            def proj_k():
                if ic == 0:
                    # w-residual term last: wk8r chunks are the last arrivals
                    k_terms = ((wk_t[0], xb8), (wk_t[0], xb8r), (wk_t[1], xb8))
                else:
                    k_terms = fb_terms(wk_t, xb8, xb8r)
                for mt in range(H_LOC):
                    ps = psA.tile([P, 512], F32, tag="psA")
                    for ti, (wt, xt) in enumerate(k_terms):
                        for a in range(NP):
                            nc.tensor.matmul(
                                ps[:],
                                lhsT=wt[:, 2 * a:2 * a + 2,
                                        mt * P:(mt + 1) * P],
                                rhs=xt[:, 2 * a:2 * a + 2, :],
                                start=(a == 0 and ti == 0),
                                stop=(a == NP - 1 and ti == 2),
                                perf_mode=mybir.MatmulPerfMode.DoubleRow)
                    if ic == 0:
                        with nc.allow_low_precision(reason="fp16 k"):
                            nc.vector.tensor_scalar(
                                out=k_res[:, mt, ts0:ts0 + 512], in0=ps[:],
                                scalar1=INV_S, scalar2=bk_sb[:, mt:mt + 1],
                                op0=mybir.AluOpType.mult,
                                op1=mybir.AluOpType.add)
                    else:
                        nc.scalar.activation(
                            k_res[:, mt, ts0:ts0 + 512], ps[:],
                            mybir.ActivationFunctionType.Identity,
                            bias=bk_sb[:, mt:mt + 1], scale=INV_S)
                    rope_inplace(k_res[:, mt, ts0:ts0 + 512],
                                 k_res[:, mt, ts0:ts0 + 512], ts0,
                                 dq=nc.scalar if ic == 0 else None)

            def proj_v():
                v_terms = ((xb8, wv_t[0]), (xb8, wv_t[1]), (xb8r, wv_t[0]))
                if ic == 0:
                    # pair-major on psS (idle before attention); term-major
                    # with the w-residual term last, since the residual
                    # weight chunks are the last DMAs to land
                    ps_v = [psA.tile([P, M], F32, tag="psA", name=f"psv{t}")
                            for t in range(2)]
                    ps_v += [psV.tile([P, M], F32, tag="psV", name=f"psv{t}")
                             for t in range(2, 4)]
                    vt0 = ((xb8, wv_t[0]), (xb8r, wv_t[0]), (xb8, wv_t[1]))
                    for ti, (xt, wt) in enumerate(vt0):
                        for a in range(NP):
                            for tt in range(4):
                                nc.tensor.matmul(
                                    ps_v[tt][:],
                                    lhsT=xt[:, 2 * a:2 * a + 2,
                                            tt * P:(tt + 1) * P],
                                    rhs=wt[:, 2 * a:2 * a + 2, :],
                                    start=(a == 0 and ti == 0),
                                    stop=(a == NP - 1 and ti == 2),
                                    perf_mode=mybir.MatmulPerfMode.DoubleRow)
                    for tt in range(4):
                        nc.vector.scalar_tensor_tensor(
                            out=v_res[:, 4 * ic + tt, :], in0=ps_v[tt][:],
                            scalar=INV_S, in1=bvb_sb[:],
                            op0=mybir.AluOpType.mult, op1=mybir.AluOpType.add)
                    return
                for tt in range(4):
                    ps = psA.tile([P, M], F32, tag="psA")
                    for ti, (xt, wt) in enumerate(v_terms):
                        for a in range(NP):
                            nc.tensor.matmul(
                                ps[:],
                                lhsT=xt[:, 2 * a:2 * a + 2,
                                        tt * P:(tt + 1) * P],
                                rhs=wt[:, 2 * a:2 * a + 2, :],
                                start=(a == 0 and ti == 0),
                                stop=(a == NP - 1 and ti == 2),
                                perf_mode=mybir.MatmulPerfMode.DoubleRow)
                    nc.vector.scalar_tensor_tensor(
                        out=v_res[:, 4 * ic + tt, :], in0=ps[:],
                        scalar=INV_S, in1=bvb_sb[:],
                        op0=mybir.AluOpType.mult, op1=mybir.AluOpType.add)

            if ic == 0:
                # wk lands last on the SP queue: fill the gap with proj v
                proj_v()
                _mark(f"v{ic}")
                proj_k()
                _mark(f"k{ic}")
            else:
                proj_k()
                _mark(f"k{ic}")
                if ic == NT - 1:
                    # att3 is Act-exp-capacity-bound while the projections
                    # are Act-idle: pull head-0's first off-diag score pairs
                    # forward so their exps overlap the v projection
                    for pj in range(4):
                        jt0, jt1 = 2 * pj, 2 * pj + 1
                        ps2p = psS.tile([P, 2, 512], F32, tag="psS2")
                        nc.tensor.matmul(
                            ps2p[:, 0, :],
                            lhsT=k_res[:, 0, jt0 * P:(jt0 + 1) * P],
                            rhs=qcur[:, 0, :], start=True, stop=True)
                        nc.tensor.matmul(
                            ps2p[:, 1, :],
                            lhsT=k_res[:, 0, jt1 * P:(jt1 + 1) * P],
                            rhs=qcur[:, 0, :], start=True, stop=True)
                        ex2p = expool.tile([P, 2, 512], F16, tag="ex")
                        nc.scalar.activation(
                            ex2p[:, :, :], ps2p[:, :, :],
                            mybir.ActivationFunctionType.Exp,
                            bias=eb_sb[:, 0:1], scale=SCALE)
                        pre_ex[pj] = ex2p
                proj_v()
                _mark(f"v{ic}")

            if ic + 1 < NT:
                x_next = load_x(ic + 1)

            if ic == 0:
                # wo is first needed by ph3(0) woven into attention(1);
                # dispatch after x1 so it doesn't steal DMA bandwidth from
                # the wv/wk/x1 loads the pipeline stalls on. Sync queue: the
                # scalar queue is head-of-line blocked on chunk-0 rope
                # shuffles; the gpsimd SWDGE path trickles.
                for j in range(H_LOC):
                    nc.sync.dma_start(out=wo_b[:, j:j + 1, :],
                                      in_=wor[:, j:j + 1, :])
                    nc.sync.dma_start(out=wor_b[:, j:j + 1, :],
                                      in_=worr[:, j:j + 1, :])

            # ---- attention for chunk ic (weaving ph3 of chunk ic-1) ----
            attn8 = atpool.tile([P, H_LOC, 512], F8, tag="attn8")
            attn8r = atpool.tile([P, H_LOC, 512], F8, tag="attn8r")
            njt = 4 * ic + 4
            slots_left = H_LOC * njt

            def emit_chain(dfr):
                """Deferred per-head softmax-normalize chain: recip + fp8
                attn split. Deferred into the NEXT head's tile loop so the
                in-order DVE queue never parks waiting for the Pool
                all_reduce (which sits behind woven ph3 evicts)."""
                rden_d, ps_av_d, hh = dfr
                with nc.allow_low_precision(reason="softmax reciprocal"):
                    if rden_d.space == tile.bass.MemorySpace.PSUM:
                        den_ps = rden_d
                        rden_d = rdpool.tile([P, 512], F16, tag="rden")
                        nc.vector.reciprocal(rden_d[:], den_ps[:, 0, :])
                    else:
                        nc.vector.reciprocal(rden_d[:], rden_d[:])
                    nc.vector.scalar_tensor_tensor(
                        out=attn8[:, hh, :], in0=ps_av_d[:], scalar=SA,
                        in1=rden_d[:],
                        op0=mybir.AluOpType.mult, op1=mybir.AluOpType.mult)
                    a16 = a16pool.tile([P, 512], F16, tag="a16")
                    nc.vector.scalar_tensor_tensor(
                        out=a16[:], in0=ps_av_d[:], scalar=SA, in1=rden_d[:],
                        op0=mybir.AluOpType.mult, op1=mybir.AluOpType.mult)
                    eng = nc.vector if hh == H_LOC - 1 else nc.gpsimd
                    eng.tensor_tensor(
                        attn8r[:, hh, :], a16[:], attn8[:, hh, :],
                        mybir.AluOpType.subtract)

            deferred = None
            ic0_chains = []
            n_items = CT * 6
            slots_done = 0
            for h in range(H_LOC):
                ps_av = psV.tile([P, 512], F32, tag="psV")
                exsum = espool.tile([P, 512], F16, tag="exsum")
                prev = None  # (ex2 tile, c00, c01, jt0) awaiting av matmuls
                for pj in range(njt // 2):
                    jt0, jt1 = 2 * pj, 2 * pj + 1
                    d0, d1 = jt0 - 4 * ic, jt1 - 4 * ic
                    c00 = 128 * d0 if d0 > 0 else 0
                    c01 = 128 * d1 if d1 > 0 else 0
                    if pj == min(2, njt // 2 - 1) and deferred is not None:
                        emit_chain(deferred)
                        deferred = None
                    if h == 0 and pj in pre_ex:
                        # precomputed during the projection phase
                        ex2 = pre_ex[pj]
                        slots_done += 2
                        if pending is not None:
                            eff = slots_left + (28 if ic == NT - 1 else 0)
                            tgt = min(n_items,
                                      (n_items * slots_done) // eff)
                            pending.step(tgt - pending.pos)
                        if prev is not None:
                            pex2, pc00, pc01, pjt0 = prev
                            nc.tensor.matmul(
                                ps_av[:, pc00:],
                                lhsT=v_res[:, pjt0, h * HS:(h + 1) * HS],
                                rhs=pex2[:, 0, pc00:],
                                start=(pjt0 == 0), stop=False,
                                skip_group_check=True)
                            nc.tensor.matmul(
                                ps_av[:, pc01:],
                                lhsT=v_res[:, pjt0 + 1, h * HS:(h + 1) * HS],
                                rhs=pex2[:, 1, pc01:],
                                start=False, stop=False,
                                skip_group_check=True)
                        with nc.allow_low_precision(reason="fp16 denom"):
                            if pj == 0:
                                nc.vector.tensor_tensor(
                                    exsum[:], ex2[:, 0, :], ex2[:, 1, :],
                                    mybir.AluOpType.add)
                            else:
                                nc.vector.tensor_tensor(
                                    exsum[:], exsum[:],
                                    ex2[:, 0, :], mybir.AluOpType.add)
                                nc.vector.tensor_tensor(
                                    exsum[:], exsum[:],
                                    ex2[:, 1, :], mybir.AluOpType.add)
                        prev = (ex2, 0, 0, jt0)
                        continue
                    # scores for a PAIR of key tiles into one 2-bank PSUM
                    # tile; one Act exp over both planes amortizes the
                    # fixed per-op Act overhead (the attention pacer)
                    ps2 = psS.tile([P, 2, 512], F32, tag="psS2")
                    nc.tensor.matmul(
                        ps2[:, 0, c00:],
                        lhsT=k_res[:, h, jt0 * P:(jt0 + 1) * P],
                        rhs=qcur[:, h, c00:],
                        start=True, stop=True)
                    split = d0 >= 0 and ic < NT - 1
                    # diagonal pairs in PE-bound windows: exact-trim plane 1
                    # and pay a second (smaller) exp on the slack Act engine.
                    # Otherwise plane 1 also starts at c00: cols [c00:c01]
                    # are real (causal-masked) scores so the shared exp
                    # never reads uninitialized PSUM
                    c1lo = c01 if split else c00
                    nc.tensor.matmul(
                        ps2[:, 1, c1lo:],
                        lhsT=k_res[:, h, jt1 * P:(jt1 + 1) * P],
                        rhs=qcur[:, h, c1lo:],
                        start=True, stop=True)
                    ex2 = expool.tile([P, 2, 512], F16, tag="ex")
                    if split:
                        nc.scalar.activation(
                            ex2[:, 0, c00:], ps2[:, 0, c00:],
                            mybir.ActivationFunctionType.Exp,
                            bias=eb_sb[:, 0:1], scale=SCALE)
                        nc.scalar.activation(
                            ex2[:, 1, c01:], ps2[:, 1, c01:],
                            mybir.ActivationFunctionType.Exp,
                            bias=eb_sb[:, 0:1], scale=SCALE)
                    else:
                        nc.scalar.activation(
                            ex2[:, :, c00:], ps2[:, :, c00:],
                            mybir.ActivationFunctionType.Exp,
                            bias=eb_sb[:, 0:1], scale=SCALE)
                    # causal triangle: for every diagonal tile the global
                    # query base ts0+c0 equals the key base jt*P, so one
                    # [P,P] keep-where-col>=row mask serves them all
                    if d0 >= 0:
                        nc.vector.tensor_tensor(
                            ex2[:, 0, c00:c00 + P], ex2[:, 0, c00:c00 + P],
                            tri_sb[:], mybir.AluOpType.mult)
                    if d1 >= 0:
                        nc.vector.tensor_tensor(
                            ex2[:, 1, c01:c01 + P], ex2[:, 1, c01:c01 + P],
                            tri_sb[:], mybir.AluOpType.mult)
                    slots_done += 2
                    if pending is not None:
                        # spread ph3 items evenly across the window; on the
                        # last chunk hold ~16 back so they bridge the final
                        # head's normalize-chain latency after the window
                        eff = slots_left + (28 if ic == NT - 1 else 0)
                        tgt = min(n_items, (n_items * slots_done) // eff)
                        pending.step(tgt - pending.pos)
                    elif qw is not None:
                        nqw = len(qw.items)
                        qw.step((nqw * slots_done) // slots_left - qw.pos)
                    if prev is not None:
                        pex2, pc00, pc01, pjt0 = prev
                        nc.tensor.matmul(
                            ps_av[:, pc00:],
                            lhsT=v_res[:, pjt0, h * HS:(h + 1) * HS],
                            rhs=pex2[:, 0, pc00:],
                            start=(pjt0 == 0), stop=False,
                            skip_group_check=True)
                        nc.tensor.matmul(
                            ps_av[:, pc01:],
                            lhsT=v_res[:, pjt0 + 1, h * HS:(h + 1) * HS],
                            rhs=pex2[:, 1, pc01:],
                            start=False, stop=False,
                            skip_group_check=True)
                    with nc.allow_low_precision(reason="fp16 softmax denom"):
                        if pj == 0:
                            if ic == 0:
                                nc.vector.tensor_copy(out=exsum[:],
                                                      in_=ex2[:, 0, :])
                                nc.vector.tensor_tensor(
                                    exsum[:, c01:], exsum[:, c01:],
                                    ex2[:, 1, c01:], mybir.AluOpType.add)
                            else:
                                nc.vector.tensor_tensor(
                                    exsum[:], ex2[:, 0, :], ex2[:, 1, :],
                                    mybir.AluOpType.add)
                        else:
                            nc.vector.tensor_tensor(
                                exsum[:, c00:], exsum[:, c00:],
                                ex2[:, 0, c00:], mybir.AluOpType.add)
                            nc.vector.tensor_tensor(
                                exsum[:, c01:], exsum[:, c01:],
                                ex2[:, 1, c01:], mybir.AluOpType.add)
                    prev = (ex2, c00, c01, jt0)
                pex2, pc00, pc01, pjt0 = prev
                nc.tensor.matmul(
                    ps_av[:, pc00:],
                    lhsT=v_res[:, pjt0, h * HS:(h + 1) * HS],
                    rhs=pex2[:, 0, pc00:],
                    start=(pjt0 == 0), stop=False,
                    skip_group_check=True)
                nc.tensor.matmul(
                    ps_av[:, pc01:],
                    lhsT=v_res[:, pjt0 + 1, h * HS:(h + 1) * HS],
                    rhs=pex2[:, 1, pc01:],
                    start=False, stop=True,
                    skip_group_check=True)
                if (h == H_LOC - 1 and ic == NT - 1) or ic == 0:
                    # chunk 0 (PE idle, DVE-bound) and end of kernel (PE
                    # free, chain gates the output projection): sum the
                    # denominator with a ones-matmul instead of the slower
                    # Pool all_reduce
                    ps_d = psS.tile([P, 2, 512], F32, tag="psS2")
                    nc.tensor.matmul(
                        ps_d[:, 0, :], lhsT=ones_sb[:], rhs=exsum[:],
                        start=True, stop=True)
                    deferred = (ps_d, ps_av, h)
                else:
                    rden = rdpool.tile([P, 512], F16, tag="rden")
                    nc.gpsimd.partition_all_reduce(
                        rden[:], exsum[:], channels=P,
                        reduce_op=bass_isa.ReduceOp.add)
                    deferred = (rden, ps_av, h)
                if h == H_LOC - 1 and ic < NT - 1:
                    emit_chain(deferred)
                    deferred = None

            _mark(f"att{ic}")
            for dfr in ic0_chains:
                emit_chain(dfr)
            if pending is not None:
                pending.finish()
            _mark(f"ph3fin{ic}")
            pending = Ph3(ic, attn8, attn8r,
                          pools=[psA, psV] if ic == NT - 1 else None)
            if deferred is not None:
                # last chunk: co0's three g0 steps only read heads 0-1; emit
                # them BEFORE the last head's normalize chain so they don't
                # inherit a wait on its attn8 write (sem counts are
                # emission-order conservative) and bridge its latency
                pending.step(6)
                emit_chain(deferred)
                deferred = None

        pending.finish()
        _mark("ph3last")

    nc.finalize()
    return nc


def get_nc(phases=(1, 2, 3)):
    if phases not in _NC_CACHE:
        _NC_CACHE[phases] = _build(phases)
    return _NC_CACHE[phases]


def _rope_tables():
    inv_freq = 1.0 / (BASE ** (np.arange(0, ROT, 2, dtype=np.float64) / ROT))
    freqs = np.arange(T, dtype=np.float64)[:, None] * inv_freq[None, :]  # [T, 32]
    cos_h = np.cos(freqs).T.astype(np.float32)   # [32, T]
    sin_h = np.sin(freqs).T.astype(np.float32)
    cosT = np.concatenate([cos_h, cos_h], axis=0)          # [64, T]
    nsT = np.concatenate([-sin_h, sin_h], axis=0)          # [64, T] signed sin
    return (np.ascontiguousarray(cosT.astype(np.float16)),
            np.ascontiguousarray(nsT.astype(np.float16)))


def _split8(a, s):
    """a*s = a8 + a8r (both fp8 e4m3) up to second-order quantization."""
    scaled = a * np.float32(s)
    a8 = scaled.astype(NPF8)
    a8r = (scaled - a8.astype(np.float32)).astype(NPF8)
    return np.ascontiguousarray(a8), np.ascontiguousarray(a8r)


def make_in_maps(x, Wq, bq, Wk, bk, Wv, bv, Wo, bo):
    cosT, nsT = _rope_tables()
    in_maps = []
    for c in range(N_CORES):
        b, g = divmod(c, TPG)
        ms = slice(g * M, (g + 1) * M)
        xh8, xh8r = _split8(x[b].T, SX)
        wq8, wq8r = _split8(Wq[ms].T, SW)
        wk8, wk8r = _split8(Wk[ms].T, SW)
        wv8, wv8r = _split8(Wv[ms].T, SW)
        wo8, wo8r = _split8(Wo[:, ms].T, SWO)
        in_maps.append({
            "xh8": xh8, "xh8r": xh8r,
            "wq8": wq8, "wq8r": wq8r,
            "wk8": wk8, "wk8r": wk8r,
            "wv8": wv8, "wv8r": wv8r,
            "wo8T": wo8, "wo8rT": wo8r,
            "bqc": np.ascontiguousarray(bq[ms].reshape(H_LOC, P).T),
            "bkc": np.ascontiguousarray(bk[ms].reshape(H_LOC, P).T),
            "bvr": np.ascontiguousarray(bv[ms].reshape(1, M).astype(np.float16)),
            "ebias": np.full((P, 1), EXP_BIAS, np.float32),
            "trimask": np.triu(np.ones((P, P), np.float16)),
            "cosT": cosT,
            "nsT": nsT,
        })
    return in_maps


def assemble(results, bo):
    out = np.empty((B, T, C), dtype=np.float32)
    for b in range(B):
        acc = results[b * TPG]["outT"].astype(np.float32)
        for g in range(1, TPG):
            acc = acc + results[b * TPG + g]["outT"].astype(np.float32)
        out[b] = acc.T + bo[None, :]
    return out


def kernel(x, Wq, bq, Wk, bk, Wv, bv, Wo, bo):
    nc = get_nc()
    in_maps = make_in_maps(np.asarray(x, np.float32),
                           np.asarray(Wq, np.float32), np.asarray(bq, np.float32),
                           np.asarray(Wk, np.float32), np.asarray(bk, np.float32),
                           np.asarray(Wv, np.float32), np.asarray(bv, np.float32),
                           np.asarray(Wo, np.float32), np.asarray(bo, np.float32))
    res = run_bass_kernel_spmd(nc, in_maps, list(range(N_CORES)))
    return assemble(res.results, np.asarray(bo, np.float32))

